# revision 31
# baseline (speedup 1.0000x reference)
"""DisentangledGNN Trainium2 kernel (8 NeuronCores, SPMD) — v2.

Strategy: target-bucketed node sharding (each core owns n/8 nodes and all
edges targeting them), with a host-side degree-balanced node permutation so
every (core, chunk) bucket holds ~equal edge counts.

Per core:
  P0  pca matmul (bf16, bias via ones-row) + leaky_relu + grouped l2norm
      (1/sqrt via exp(-0.5*ln(x)) so the Act engine never switches
      activation tables away from the exp/ln set)
  P1  AllGather of normalized features, split into 4 sub-collectives
      overlapped under P0
  P2  z = Hp[src] edge gather (indirect DMA, batched 4 tiles/instruction)
  P3  3 routing iterations; per 128-edge tile the u[trg] gather and the
      segment-sum scatter are one-hot matmuls whose fp8 mask matrices are
      precomputed on host and streamed via DMA.  Softmax over the 10
      factors: exp on Act, sums/reciprocal on DVE, and the p-broadcast to
      dd=16 via a bf16-pair trick (each p duplicated into a bf16 pair,
      bitcast f32, broadcast x8 on Act = half the elements).
  P4  (last iteration) leaky_relu + classifier matmul, bias via ones-row.
No inter-core communication during routing.
"""

import numpy as np
import ml_dtypes

import concourse.bass as bass
import concourse.mybir as mybir
import concourse.tile as tile
from concourse.masks import make_identity
from concourse.bass_utils import run_bass_kernel_spmd

F32 = mybir.dt.float32
BF16 = mybir.dt.bfloat16
I32 = mybir.dt.int32
FP8 = mybir.dt.float8e4
AF = mybir.ActivationFunctionType
AX = mybir.AxisListType
OP = mybir.AluOpType

K = 10
SLOPE = 0.01
NITER = 3
P = 128
ZBATCH = 1   # tiles per indirect-DMA gather (HW SWDGE only honors [P,1] offsets)
ZBUFS = 24   # chunks of z kept in SBUF (prefetch window)


def _split_multiwaits(nc):
    # This walrus accepts at most 1 sync wait per instruction (2 for
    # EventSemaphore ops); split extras onto preceding same-engine NOPs.
    n = [0]
    for fn in nc.m.functions:
        for blk in fn.blocks:
            newinsts = []
            changed = False
            for ins in blk.instructions:
                si = ins.sync_info
                cap = 2 if "EventSem" in type(ins).__name__ else 1
                if si is not None and len(si.on_wait) > cap:
                    waits = list(si.on_wait)
                    for w in waits[cap:]:
                        n[0] += 1
                        nop = mybir.InstNoOp(name=f"{ins.name}-ws{n[0]}", ins=[], outs=[])
                        nop.engine = ins.engine
                        nop.sync_info = mybir.SyncInfo(on_wait=[w], on_update=[])
                        newinsts.append(nop)
                    si.on_wait = waits[:cap]
                    ins.sync_info = si
                    changed = True
                newinsts.append(ins)
            if changed:
                blk.instructions = newinsts


def _host_prep(x, edge_index, n_cores):
    """Degree-balanced node->(core,chunk,slot) assignment, edge bucketing,
    fp8 one-hot mask matrices, permuted bf16 xT, Hp row mapping."""
    n, nfeat = x.shape
    npc = n // n_cores
    nchunks = (npc + P - 1) // P
    npc_pad = nchunks * P
    src = np.asarray(edge_index[0], np.int64)
    trg = np.asarray(edge_index[1], np.int64)

    deg = np.bincount(trg, minlength=n).astype(np.int64)

    # Greedy: nodes in descending-degree order to the (core,chunk) bin with
    # the fewest edges, subject to <=128 nodes/bin and npc nodes/core.
    order = np.argsort(-deg, kind="stable")
    bin_edges = np.zeros((n_cores, nchunks), np.int64)
    bin_nodes = np.zeros((n_cores, nchunks), np.int64)
    core_nodes = np.zeros(n_cores, np.int64)
    node_core = np.empty(n, np.int32)
    node_chunk = np.empty(n, np.int32)
    node_slot = np.empty(n, np.int32)
    INF = 1 << 60
    for nd in order:
        feas = (bin_nodes < P) & (core_nodes[:, None] < npc)
        masked = np.where(feas, bin_edges, INF)
        ci = int(np.argmin(masked))
        c, j = divmod(ci, nchunks)
        node_core[nd] = c
        node_chunk[nd] = j
        node_slot[nd] = bin_nodes[c, j]
        bin_nodes[c, j] += 1
        core_nodes[c] += 1
        bin_edges[c, j] += deg[nd]

    nt = np.maximum(1, (bin_edges.max(axis=0) + P - 1) // P).astype(np.int64)
    T = int(nt.sum())
    tile_of_chunk = np.concatenate([[0], np.cumsum(nt)]).astype(np.int64)

    # AllGather split points (chunk granularity) and Hp row mapping.
    nsplit = min(2, nchunks)
    bounds = [round(q * nchunks / nsplit) for q in range(nsplit + 1)]
    rows_q = [(bounds[q + 1] - bounds[q]) * P for q in range(nsplit)]
    hq_base = np.concatenate([[0], np.cumsum([n_cores * r for r in rows_q])])
    pos_in_core = node_chunk * P + node_slot
    node_split = np.searchsorted(np.asarray(bounds[1:]) * P, pos_in_core, side="right")
    hp_row = (
        hq_base[node_split]
        + node_core * np.asarray(rows_q)[node_split]
        + (pos_in_core - np.asarray(bounds)[node_split] * P)
    ).astype(np.int32)

    # Edge bucketing per core, chunk-sorted; slots padded with lloc=255.
    e_core = node_core[trg]
    e_chunk = node_chunk[trg]
    e_lloc = node_slot[trg]
    e_srow = hp_row[src]
    eorder = np.lexsort((e_lloc, e_chunk, e_core))
    e_core, e_chunk, e_lloc, e_srow = (
        e_core[eorder], e_chunk[eorder], e_lloc[eorder], e_srow[eorder])

    src_arr = np.zeros((n_cores, T * P), np.int32)
    lloc_arr = np.full((n_cores, T * P), 255, np.int32)
    core_starts = np.searchsorted(e_core, np.arange(n_cores + 1))
    for c in range(n_cores):
        cs, ce = core_starts[c], core_starts[c + 1]
        chunk_c = e_chunk[cs:ce]
        starts = np.searchsorted(chunk_c, np.arange(nchunks + 1))
        for j in range(nchunks):
            e0, e1 = cs + starts[j], cs + starts[j + 1]
            base = int(tile_of_chunk[j]) * P
            cnt = e1 - e0
            src_arr[c, base:base + cnt] = e_srow[e0:e1]
            lloc_arr[c, base:base + cnt] = e_lloc[e0:e1]

    # Device layouts: slot s -> tile s//P, lane s%P  => [P, T]
    src_dev = src_arr.reshape(n_cores, T, P).transpose(0, 2, 1).copy()
    lloc_mat = lloc_arr.reshape(n_cores, T, P).transpose(0, 2, 1)  # [c, P, T]

    # fp8 one-hot masks.  S[e-lane, t, v] = (lloc==v); ST is per-tile transpose.
    ar = np.arange(P)
    S_bool = lloc_mat[:, :, :, None] == ar[None, None, None, :]     # [c,P,T,128]
    ST_bool = S_bool.transpose(0, 3, 2, 1)                          # [c,P,T,128]
    S_dev = S_bool.astype(ml_dtypes.float8_e4m3fn).reshape(n_cores, P, T * P)
    ST_dev = np.ascontiguousarray(ST_bool).astype(ml_dtypes.float8_e4m3fn).reshape(n_cores, P, T * P)

    # Permuted xT in bf16, ones row for the pca bias.
    kf_pad = ((nfeat + 1 + P - 1) // P) * P
    xT = np.zeros((n_cores, kf_pad, npc_pad), ml_dtypes.bfloat16)
    xb = x.astype(ml_dtypes.bfloat16)
    for c in range(n_cores):
        nodes_c = np.where(node_core == c)[0]
        xT[c][:nfeat, pos_in_core[nodes_c]] = xb[nodes_c].T
    xT[:, nfeat, :] = 1.0

    meta = dict(npc=npc, nchunks=nchunks, npc_pad=npc_pad, nt=nt, T=T,
                tile_of_chunk=tile_of_chunk, bounds=bounds, rows_q=rows_q,
                hq_base=hq_base, kf_pad=kf_pad,
                node_core=node_core, pos_in_core=pos_in_core)
    return meta, src_dev, S_dev, ST_dev, xT


GT = 6  # tiles per vector group (2 PSUM banks x 3 tiles)


def _group_plan(ntj):
    """Split a chunk's ntj tiles into vector groups over the 3-bank ut
    supertile.  Returns list of (g0, gn, spans, (nfull, rem)) where spans
    are F32-element offsets into the [P,1536] supertile; the Act copy is
    one instruction over nfull full banks plus one for the remainder."""
    plan = []
    g0 = 0
    while g0 < ntj:
        gn = min(GT, ntj - g0)
        nfull, rem = divmod(gn, 3)
        spans = [512 * b + 160 * i for b in range(nfull) for i in range(3)]
        spans += [512 * nfull + 160 * i for i in range(rem)]
        plan.append((g0, gn, spans, (nfull, rem)))
        g0 += gn
    return plan


def build_program(nfeat, d, nclass, meta, n_cores):
    dd = d // K
    npc_pad = meta["npc_pad"]
    nchunks = meta["nchunks"]
    nt = meta["nt"]
    T = meta["T"]
    toc = meta["tile_of_chunk"]
    bounds = meta["bounds"]
    rows_q = meta["rows_q"]
    hq_base = meta["hq_base"]
    kf_pad = meta["kf_pad"]
    nkt = kf_pad // P
    HROWS = int(hq_base[-1])
    max_nt = int(nt.max())

    nc = bass.Bass(num_devices=n_cores)

    xT_t = nc.dram_tensor("xT", [kf_pad, npc_pad], BF16, kind="ExternalInput")
    w_t = nc.dram_tensor("wp", [kf_pad, d], BF16, kind="ExternalInput")
    cw_t = nc.dram_tensor("cwp", [P, 3 * nclass], BF16, kind="ExternalInput")
    src_t = nc.dram_tensor("src", [P, T], I32, kind="ExternalInput")
    S_t = nc.dram_tensor("Smask", [P, T * P], FP8, kind="ExternalInput")
    ST_t = nc.dram_tensor("STmask", [P, T * P], FP8, kind="ExternalInput")
    y_t = nc.dram_tensor("y", [npc_pad, nclass], F32, kind="ExternalOutput")
    Hp = nc.dram_tensor("Hp", [HROWS, d], BF16, kind="Internal")

    with tile.TileContext(nc) as tc:
        with (
            tc.tile_pool(name="persist", bufs=1) as pp,
            tc.tile_pool(name="dram", bufs=1, space="DRAM") as dp,
            tc.tile_pool(name="p0", bufs=2) as sb,
            tc.tile_pool(name="mask", bufs=2) as sm,
            tc.tile_pool(name="zpool", bufs=ZBUFS) as sz,
            tc.tile_pool(name="ring", bufs=2) as sr,
            tc.tile_pool(name="epi", bufs=2) as se,
            tc.tile_pool(name="put", bufs=2, space="PSUM") as put,
            tc.tile_pool(name="pseg", bufs=1, space="PSUM") as pse,
            tc.tile_pool(name="ptr", bufs=1, space="PSUM") as ptr,
        ):
            # ---------------- constants / persistent state ----------------
            ident = pp.tile([P, P], BF16)
            make_identity(nc, ident[:])
            ones_sb = pp.tile([1, P], BF16)
            nc.vector.memset(ones_sb[:], 1.0)
            eps_b = pp.tile([P, 1], F32)
            nc.vector.memset(eps_b[:], 1e-24)

            w_sb = pp.tile([P, nkt * d], BF16)
            nc.sync.dma_start(
                out=w_sb[:].rearrange("p (a q) -> p a q", q=d),
                in_=w_t[:].rearrange("(a p) q -> p a q", p=P),
            )
            cw_sb = pp.tile([P, 3 * nclass], BF16)
            nc.sync.dma_start(out=cw_sb[:], in_=cw_t[:])
            src_sb = pp.tile([P, T], I32)
            nc.sync.dma_start(out=src_sb[:], in_=src_t[:])

            hn = pp.tile([P, nchunks * d], BF16)  # normalized features (own nodes)
            ag_in = dp.tile([npc_pad, d], BF16)

            # ---------------- P0: pca + lrelu + l2norm + sub-allgathers ----
            qnext = 0
            for m in range(nchunks):
                xt = sb.tile([P, nkt * P], BF16, tag="xt", bufs=3)
                nc.sync.dma_start(
                    out=xt[:].rearrange("p (a q) -> p a q", q=P),
                    in_=xT_t[:, m * P:(m + 1) * P].rearrange("(a p) q -> p a q", p=P),
                )
                h_ps = put.tile([P, 1024], F32, space="PSUM", tag="ut")
                for a in range(nkt):
                    nc.tensor.matmul(
                        out=h_ps[:, :d],
                        lhsT=xt[:, a * P:(a + 1) * P],
                        rhs=w_sb[:, a * d:(a + 1) * d],
                        start=(a == 0),
                        stop=(a == nkt - 1),
                    )
                hs = sb.tile([P, d], F32, tag="hs")
                nc.vector.tensor_scalar_mul(out=hs[:], in0=h_ps[:, :d], scalar1=SLOPE)
                h = sb.tile([P, d], F32, tag="h")
                nc.vector.tensor_tensor(out=h[:], in0=h_ps[:, :d], in1=hs[:], op=OP.max)
                sq = sb.tile([P, d], F32, tag="sq")
                nc.scalar.activation(out=sq[:], in_=h[:], func=AF.Square)
                ss = sb.tile([P, K], F32, tag="ss")
                nc.vector.reduce_sum(
                    out=ss[:], in_=sq[:].rearrange("p (k e) -> p k e", k=K),
                    axis=AX.X,
                )
                lg = sb.tile([P, K], F32, tag="lg")
                nc.scalar.activation(out=lg[:], in_=ss[:], func=AF.Ln, bias=eps_b[:, :1])
                rr = sb.tile([P, K], F32, tag="rr")
                nc.scalar.activation(out=rr[:], in_=lg[:], func=AF.Exp, scale=-0.5)
                nc.vector.tensor_tensor(
                    out=hn[:, m * d:(m + 1) * d].rearrange("p (k e) -> p k e", k=K),
                    in0=h[:].rearrange("p (k e) -> p k e", k=K),
                    in1=rr[:].unsqueeze(2).to_broadcast([P, K, dd]),
                    op=OP.mult,
                )
                nc.sync.dma_start(
                    out=ag_in[m * P:(m + 1) * P, :], in_=hn[:, m * d:(m + 1) * d]
                )
                if m == bounds[qnext + 1] - 1:
                    q = qnext
                    nc.gpsimd.collective_compute(
                        "AllGather",
                        OP.bypass,
                        replica_groups=[list(range(n_cores))],
                        ins=[ag_in[bounds[q] * P:bounds[q + 1] * P, :]],
                        outs=[Hp.ap()[int(hq_base[q]):int(hq_base[q + 1]), :]],
                    )
                    qnext += 1

            # ---------------- P3: routing ---------------------------------
            def chunk_prologue(j):
                t0, ntj = int(toc[j]), int(nt[j])
                S_sb = sm.tile([P, max_nt * P], FP8, tag=f"S{j % 3}")
                nc.sync.dma_start(
                    out=S_sb[:, :ntj * P], in_=S_t[:, t0 * P:(t0 + ntj) * P]
                )
                ST_sb = sm.tile([P, max_nt * P], FP8, tag=f"ST{j % 3}")
                nc.sync.dma_start(
                    out=ST_sb[:, :ntj * P], in_=ST_t[:, t0 * P:(t0 + ntj) * P]
                )
                zch = sz.tile([P, max_nt * d], BF16, tag="z")
                for b0 in range(0, ntj, ZBATCH):
                    bn = min(ZBATCH, ntj - b0)
                    nc.gpsimd.indirect_dma_start(
                        out=zch[:, b0 * d:(b0 + bn) * d],
                        out_offset=None,
                        in_=Hp.ap(),
                        in_offset=bass.IndirectOffsetOnAxis(
                            ap=src_sb[:, t0 + b0:t0 + b0 + bn], axis=0
                        ),
                    )
                return dict(j=j, ntj=ntj, zch=zch, S_sb=S_sb, ST_sb=ST_sb,
                            u_j=None, plan=_group_plan(ntj))

            def emit_group(st, it, plan_entry):
                j, zch, S_sb, ST_sb, seg = (
                    st["j"], st["zch"], st["S_sb"], st["ST_sb"], st["seg"])
                hn_j = hn[:, j * d:(j + 1) * d]
                u_rhs = hn_j if it == 0 else st["u_j"][:]
                ti = st["ti"]
                for (g0, gn, spans, (nfull, rem)) in [plan_entry]:
                    utp = put.tile([P, 1024], F32, space="PSUM", tag="ut")
                    for i, t in enumerate(range(g0, g0 + gn)):
                        nc.tensor.matmul(
                            out=utp[:, spans[i]:spans[i] + d],
                            lhsT=ST_sb[:, t * P:(t + 1) * P],
                            rhs=u_rhs,
                            start=True, stop=True,
                        )
                    utb = sr.tile([P, GT * d], BF16, tag="utb")
                    if nfull:
                        nc.scalar.copy(
                            out=utb[:, :nfull * 3 * d],
                            in_=utp[:, :nfull * 512].rearrange(
                                "p (b x) -> p b x", b=nfull)[:, :, :3 * d],
                        )
                    if rem:
                        nc.scalar.copy(
                            out=utb[:, nfull * 3 * d:gn * d],
                            in_=utp[:, nfull * 512:nfull * 512 + rem * d],
                        )
                    zg = zch[:, g0 * d:(g0 + gn) * d]
                    prod = sr.tile([P, GT * d], BF16, tag="prod")
                    nc.vector.tensor_mul(out=prod[:, :gn * d], in0=zg, in1=utb[:, :gn * d])
                    pv = prod[:, :gn * d].rearrange("p (a e) -> p a e", e=dd)
                    t1 = sr.tile([P, GT * d // 2], BF16, tag="t1")
                    nc.vector.tensor_add(
                        out=t1[:, :gn * d // 2].rearrange("p (a e) -> p a e", e=8),
                        in0=pv[:, :, 0:8], in1=pv[:, :, 8:16],
                    )
                    t1v = t1[:, :gn * d // 2].rearrange("p (a e) -> p a e", e=8)
                    t2 = sr.tile([P, GT * d // 4], BF16, tag="t2")
                    nc.vector.tensor_add(
                        out=t2[:, :gn * d // 4].rearrange("p (a e) -> p a e", e=4),
                        in0=t1v[:, :, 0:4], in1=t1v[:, :, 4:8],
                    )
                    t2v = t2[:, :gn * d // 4].rearrange("p (a e) -> p a e", e=4)
                    t3 = sr.tile([P, GT * d // 8], BF16, tag="t3")
                    nc.vector.tensor_add(
                        out=t3[:, :gn * d // 8].rearrange("p (a e) -> p a e", e=2),
                        in0=t2v[:, :, 0:2], in1=t2v[:, :, 2:4],
                    )
                    t3v = t3[:, :gn * d // 8].rearrange("p (a e) -> p a e", e=2)
                    sf = sr.tile([P, GT * K], F32, tag="sf")
                    nc.vector.tensor_add(
                        out=sf[:, :gn * K],
                        in0=t3v[:, :, 0:1].squeeze(2), in1=t3v[:, :, 1:2].squeeze(2),
                    )
                    ef = sr.tile([P, GT * K], BF16, tag="ef")
                    nc.scalar.activation(out=ef[:, :gn * K], in_=sf[:, :gn * K], func=AF.Exp)
                    qf = sr.tile([P, GT], F32, tag="qf")
                    nc.vector.reduce_sum(
                        out=qf[:, :gn],
                        in_=ef[:, :gn * K].rearrange("p (a k) -> p a k", k=K),
                        axis=AX.X,
                    )
                    rf = sr.tile([P, GT], F32, tag="rf")
                    nc.vector.reciprocal(out=rf[:, :gn], in_=qf[:, :gn])
                    pe2 = sr.tile([P, GT * K * 2], BF16, tag="pe2")
                    p2v = pe2[:].rearrange("p (a k two) -> p a k two", k=K, two=2)
                    efv = ef[:, :gn * K].rearrange("p (a k) -> p a k", k=K)
                    rfv = rf[:, :gn].unsqueeze(2).to_broadcast([P, gn, K])
                    nc.vector.tensor_tensor(
                        out=p2v[:, :gn, :, 0:1].squeeze(3), in0=efv, in1=rfv, op=OP.mult)
                    nc.vector.tensor_tensor(
                        out=p2v[:, :gn, :, 1:2].squeeze(3), in0=efv, in1=rfv, op=OP.mult)
                    pex = sr.tile([P, GT * d], BF16, tag="pex")
                    nc.scalar.copy(
                        out=pex.bitcast(F32)[:, :gn * d // 2].rearrange(
                            "p (a e) -> p a e", e=dd // 2),
                        in_=pe2.bitcast(F32)[:, :gn * K].unsqueeze(2).to_broadcast(
                            [P, gn * K, dd // 2]),
                    )
                    msg = sr.tile([P, GT * d], BF16, tag="msg")
                    nc.vector.tensor_mul(out=msg[:, :gn * d], in0=zg, in1=pex[:, :gn * d])
                    for i, t in enumerate(range(g0, g0 + gn)):
                        nc.tensor.matmul(
                            out=seg,
                            lhsT=S_sb[:, t * P:(t + 1) * P],
                            rhs=msg[:, i * d:(i + 1) * d],
                            start=(ti == 0), stop=False,
                        )
                        ti += 1
                st["ti"] = ti

            def chunk_residual(st):
                # + x residual via identity matmul, closes the accumulation
                nc.tensor.matmul(
                    out=st["seg"], lhsT=ident[:],
                    rhs=hn[:, st["j"] * d:(st["j"] + 1) * d],
                    start=False, stop=True)

            def chunk_epilogue(st, it):
                j, seg = st["j"], st["seg"]
                sq2 = se.tile([P, d], F32, tag="sq2")
                nc.scalar.activation(out=sq2[:], in_=seg, func=AF.Square)
                ss2 = se.tile([P, K], F32, tag="ss2")
                nc.vector.reduce_sum(
                    out=ss2[:], in_=sq2[:].rearrange("p (k e) -> p k e", k=K),
                    axis=AX.X,
                )
                lg2 = se.tile([P, K], F32, tag="lg2")
                nc.scalar.activation(out=lg2[:], in_=ss2[:], func=AF.Ln, bias=eps_b[:, :1])
                rr2 = se.tile([P, K], F32, tag="rr2")
                nc.scalar.activation(out=rr2[:], in_=lg2[:], func=AF.Exp, scale=-0.5)
                rrb = rr2[:].unsqueeze(2).to_broadcast([P, K, dd])
                segv = seg.rearrange("p (k e) -> p k e", k=K)
                if it < NITER - 1:
                    u_new = se.tile([P, d], BF16, tag=f"uj{j % 3}", bufs=1)
                    nc.vector.tensor_tensor(
                        out=u_new[:].rearrange("p (k e) -> p k e", k=K),
                        in0=segv, in1=rrb, op=OP.mult)
                    st["u_j"] = u_new
                else:
                    uf = se.tile([P, d], F32, tag="uf")
                    nc.vector.tensor_tensor(
                        out=uf[:].rearrange("p (k e) -> p k e", k=K),
                        in0=segv, in1=rrb, op=OP.mult)
                    us = se.tile([P, d], F32, tag="us")
                    nc.vector.tensor_scalar_mul(out=us[:], in0=uf[:], scalar1=SLOPE)
                    ufb = se.tile([P, d], BF16, tag="ufb")
                    nc.vector.tensor_tensor(out=ufb[:], in0=uf[:], in1=us[:], op=OP.max)
                    trp = ptr.tile([P, 1024], BF16, space="PSUM", tag="tr")
                    nc.tensor.transpose(out=trp[:, :P], in_=ufb[:, :P], identity=ident[:])
                    nc.tensor.transpose(
                        out=trp[:d - P, P:2 * P], in_=ufb[:, P:d], identity=ident[:])
                    uT = se.tile([P, 2 * P], BF16, tag="uT")
                    nc.scalar.copy(out=uT[:, :P], in_=trp[:, :P])
                    nc.scalar.copy(out=uT[:d - P, P:], in_=trp[:d - P, P:2 * P])
                    yp = ptr.tile([P, 1024], BF16, space="PSUM", tag="tr")
                    ypv = yp.bitcast(F32)[:, :nclass]
                    nc.tensor.matmul(
                        out=ypv, lhsT=uT[:, :P], rhs=cw_sb[:, :nclass],
                        start=True, stop=False)
                    nc.tensor.matmul(
                        out=ypv, lhsT=uT[:d - P, P:2 * P],
                        rhs=cw_sb[:d - P, nclass:2 * nclass],
                        start=False, stop=False)
                    nc.tensor.matmul(
                        out=ypv, lhsT=ones_sb[:, :P], rhs=cw_sb[0:1, 2 * nclass:],
                        start=False, stop=True)
                    ysb = se.tile([P, nclass], F32, tag="ysb")
                    nc.scalar.copy(out=ysb[:], in_=ypv)
                    nc.sync.dma_start(out=y_t[j * P:(j + 1) * P, :], in_=ysb[:])

            # full triples, then singles for the tail chunks so the pipeline
            # drain after the final z-gather is one chunk, not three
            nsing = min(4, nchunks % 3 + 3) if nchunks > 6 else 0
            while (nchunks - nsing) % 3:
                nsing += 1
            cgroups = [list(range(j0, j0 + 3))
                       for j0 in range(0, nchunks - nsing, 3)]
            cgroups += [[j] for j in range(nchunks - nsing, nchunks)]
            for cg in cgroups:
                sts = [chunk_prologue(j) for j in cg]
                for it in range(NITER):
                    for st in sts:
                        segt = pse.tile([P, 512], F32, space="PSUM",
                                        tag=f"seg{st['j'] % 3}")
                        st["seg"] = segt[:, 0:d]
                        st["ti"] = 0
                    ng = max(len(st["plan"]) for st in sts)
                    for g in range(ng):
                        for st in sts:
                            if g < len(st["plan"]):
                                emit_group(st, it, st["plan"][g])
                    for st in sts:
                        chunk_residual(st)
                    for st in sts:
                        chunk_epilogue(st, it)
    return nc


_CACHE = {}
TRACE = False
LAST_RESULTS = None


def kernel(x, edge_index, pca_w, pca_b, clf_w, clf_b, n_cores=8, _sim=False):
    x = np.asarray(x, np.float32)
    edge_index = np.asarray(edge_index)
    pca_w = np.asarray(pca_w, np.float32)
    pca_b = np.asarray(pca_b, np.float32)
    clf_w = np.asarray(clf_w, np.float32)
    clf_b = np.asarray(clf_b, np.float32)

    n, nfeat = x.shape
    d = pca_w.shape[1]
    nclass = clf_w.shape[1]

    meta, src_dev, S_dev, ST_dev, xT = _host_prep(x, edge_index, n_cores)

    key = (n, nfeat, d, nclass, tuple(meta["nt"].tolist()))
    if key not in _CACHE:
        _CACHE[key] = build_program(nfeat, d, nclass, meta, n_cores)
        if not _sim:
            _split_multiwaits(_CACHE[key])
    nc = _CACHE[key]

    kf_pad = meta["kf_pad"]
    w_pad = np.zeros((kf_pad, d), ml_dtypes.bfloat16)
    w_pad[:nfeat] = pca_w.astype(ml_dtypes.bfloat16)
    w_pad[nfeat] = pca_b.astype(ml_dtypes.bfloat16)
    cwp = np.zeros((P, 3 * nclass), ml_dtypes.bfloat16)
    cwp[:, :nclass] = clf_w[:P].astype(ml_dtypes.bfloat16)
    cwp[:d - P, nclass:2 * nclass] = clf_w[P:].astype(ml_dtypes.bfloat16)
    cwp[0, 2 * nclass:] = clf_b.astype(ml_dtypes.bfloat16)

    in_maps = []
    for c in range(n_cores):
        in_maps.append({
            "xT": xT[c],
            "wp": w_pad,
            "cwp": cwp,
            "src": src_dev[c],
            "Smask": S_dev[c],
            "STmask": ST_dev[c],
        })

    npc = meta["npc"]
    npc_pad = meta["npc_pad"]
    if _sim:
        from concourse.bass_interp import CoreSim
        assert n_cores == 1
        sim = CoreSim(nc)
        for kk, vv in in_maps[0].items():
            sim.tensor(kk)[:] = vv
        sim.simulate()
        y_dev = np.asarray(sim.tensor("y"))[None]
    else:
        global LAST_RESULTS
        res = run_bass_kernel_spmd(
            nc, in_maps, core_ids=list(range(n_cores)), trace=TRACE
        )
        LAST_RESULTS = res
        y_dev = np.stack([res.results[c]["y"] for c in range(n_cores)], axis=0)

    # un-permute: node nd lives at (core, pos)
    y = np.empty((n, nclass), np.float32)
    y[np.arange(n)] = y_dev[meta["node_core"], meta["pos_in_core"]]
    return y.astype(np.float32)


if __name__ == "__main__":
    import pickle, time
    with open("/tmp/ref_inputs.pkl", "rb") as f:
        inputs = pickle.load(f)
    t0 = time.time()
    y = kernel(**inputs)
    print("kernel() wall time", time.time() - t0)
    np.save("/tmp/kernel_out.npy", y)


# revision 33
# speedup vs baseline: 1.0127x; 1.0127x over previous
"""DisentangledGNN Trainium2 kernel (8 NeuronCores, SPMD) — v2.

Strategy: target-bucketed node sharding (each core owns n/8 nodes and all
edges targeting them), with a host-side degree-balanced node permutation so
every (core, chunk) bucket holds ~equal edge counts.

Per core:
  P0  pca matmul (bf16, bias via ones-row) + leaky_relu + grouped l2norm
      (1/sqrt via exp(-0.5*ln(x)) so the Act engine never switches
      activation tables away from the exp/ln set)
  P1  AllGather of normalized features, split into 4 sub-collectives
      overlapped under P0
  P2  z = Hp[src] edge gather (indirect DMA, batched 4 tiles/instruction)
  P3  3 routing iterations; per 128-edge tile the u[trg] gather and the
      segment-sum scatter are one-hot matmuls whose fp8 mask matrices are
      precomputed on host and streamed via DMA.  Softmax over the 10
      factors: exp on Act, sums/reciprocal on DVE, and the p-broadcast to
      dd=16 via a bf16-pair trick (each p duplicated into a bf16 pair,
      bitcast f32, broadcast x8 on Act = half the elements).
  P4  (last iteration) leaky_relu + classifier matmul, bias via ones-row.
No inter-core communication during routing.
"""

import numpy as np
import ml_dtypes

import concourse.bass as bass
import concourse.mybir as mybir
import concourse.tile as tile
from concourse.masks import make_identity
from concourse.bass_utils import run_bass_kernel_spmd

F32 = mybir.dt.float32
BF16 = mybir.dt.bfloat16
I32 = mybir.dt.int32
FP8 = mybir.dt.float8e4
AF = mybir.ActivationFunctionType
AX = mybir.AxisListType
OP = mybir.AluOpType

K = 10
SLOPE = 0.01
NITER = 3
P = 128
ZBATCH = 1   # tiles per indirect-DMA gather (HW SWDGE only honors [P,1] offsets)
ZBUFS = 24   # chunks of z kept in SBUF (prefetch window)


def _split_multiwaits(nc):
    # This walrus accepts at most 1 sync wait per instruction (2 for
    # EventSemaphore ops); split extras onto preceding same-engine NOPs.
    n = [0]
    for fn in nc.m.functions:
        for blk in fn.blocks:
            newinsts = []
            changed = False
            for ins in blk.instructions:
                si = ins.sync_info
                cap = 2 if "EventSem" in type(ins).__name__ else 1
                if si is not None and len(si.on_wait) > cap:
                    waits = list(si.on_wait)
                    for w in waits[cap:]:
                        n[0] += 1
                        nop = mybir.InstNoOp(name=f"{ins.name}-ws{n[0]}", ins=[], outs=[])
                        nop.engine = ins.engine
                        nop.sync_info = mybir.SyncInfo(on_wait=[w], on_update=[])
                        newinsts.append(nop)
                    si.on_wait = waits[:cap]
                    ins.sync_info = si
                    changed = True
                newinsts.append(ins)
            if changed:
                blk.instructions = newinsts


def _host_prep(x, edge_index, n_cores):
    """Degree-balanced node->(core,chunk,slot) assignment, edge bucketing,
    fp8 one-hot mask matrices, permuted bf16 xT, Hp row mapping."""
    n, nfeat = x.shape
    npc = n // n_cores
    nchunks = (npc + P - 1) // P
    npc_pad = nchunks * P
    src = np.asarray(edge_index[0], np.int64)
    trg = np.asarray(edge_index[1], np.int64)

    deg = np.bincount(trg, minlength=n).astype(np.int64)

    # Greedy: nodes in descending-degree order to the (core,chunk) bin with
    # the fewest edges, subject to <=128 nodes/bin and npc nodes/core.
    order = np.argsort(-deg, kind="stable")
    bin_edges = np.zeros((n_cores, nchunks), np.int64)
    bin_nodes = np.zeros((n_cores, nchunks), np.int64)
    core_nodes = np.zeros(n_cores, np.int64)
    node_core = np.empty(n, np.int32)
    node_chunk = np.empty(n, np.int32)
    node_slot = np.empty(n, np.int32)
    INF = 1 << 60
    for nd in order:
        feas = (bin_nodes < P) & (core_nodes[:, None] < npc)
        masked = np.where(feas, bin_edges, INF)
        ci = int(np.argmin(masked))
        c, j = divmod(ci, nchunks)
        node_core[nd] = c
        node_chunk[nd] = j
        node_slot[nd] = bin_nodes[c, j]
        bin_nodes[c, j] += 1
        core_nodes[c] += 1
        bin_edges[c, j] += deg[nd]

    nt = np.maximum(1, (bin_edges.max(axis=0) + P - 1) // P).astype(np.int64)
    T = int(nt.sum())
    tile_of_chunk = np.concatenate([[0], np.cumsum(nt)]).astype(np.int64)

    # AllGather split points (chunk granularity) and Hp row mapping.
    nsplit = 1
    bounds = [round(q * nchunks / nsplit) for q in range(nsplit + 1)]
    rows_q = [(bounds[q + 1] - bounds[q]) * P for q in range(nsplit)]
    hq_base = np.concatenate([[0], np.cumsum([n_cores * r for r in rows_q])])
    pos_in_core = node_chunk * P + node_slot
    node_split = np.searchsorted(np.asarray(bounds[1:]) * P, pos_in_core, side="right")
    hp_row = (
        hq_base[node_split]
        + node_core * np.asarray(rows_q)[node_split]
        + (pos_in_core - np.asarray(bounds)[node_split] * P)
    ).astype(np.int32)

    # Edge bucketing per core, chunk-sorted; slots padded with lloc=255.
    e_core = node_core[trg]
    e_chunk = node_chunk[trg]
    e_lloc = node_slot[trg]
    e_srow = hp_row[src]
    eorder = np.lexsort((e_lloc, e_chunk, e_core))
    e_core, e_chunk, e_lloc, e_srow = (
        e_core[eorder], e_chunk[eorder], e_lloc[eorder], e_srow[eorder])

    src_arr = np.zeros((n_cores, T * P), np.int32)
    lloc_arr = np.full((n_cores, T * P), 255, np.int32)
    core_starts = np.searchsorted(e_core, np.arange(n_cores + 1))
    for c in range(n_cores):
        cs, ce = core_starts[c], core_starts[c + 1]
        chunk_c = e_chunk[cs:ce]
        starts = np.searchsorted(chunk_c, np.arange(nchunks + 1))
        for j in range(nchunks):
            e0, e1 = cs + starts[j], cs + starts[j + 1]
            base = int(tile_of_chunk[j]) * P
            cnt = e1 - e0
            src_arr[c, base:base + cnt] = e_srow[e0:e1]
            lloc_arr[c, base:base + cnt] = e_lloc[e0:e1]

    # Device layouts: slot s -> tile s//P, lane s%P  => [P, T]
    src_dev = src_arr.reshape(n_cores, T, P).transpose(0, 2, 1).copy()
    lloc_mat = lloc_arr.reshape(n_cores, T, P).transpose(0, 2, 1)  # [c, P, T]

    # fp8 one-hot masks.  S[e-lane, t, v] = (lloc==v); ST is per-tile transpose.
    ar = np.arange(P)
    S_bool = lloc_mat[:, :, :, None] == ar[None, None, None, :]     # [c,P,T,128]
    ST_bool = S_bool.transpose(0, 3, 2, 1)                          # [c,P,T,128]
    S_dev = S_bool.astype(ml_dtypes.float8_e4m3fn).reshape(n_cores, P, T * P)
    ST_dev = np.ascontiguousarray(ST_bool).astype(ml_dtypes.float8_e4m3fn).reshape(n_cores, P, T * P)

    # Permuted xT in bf16, ones row for the pca bias.
    kf_pad = ((nfeat + 1 + P - 1) // P) * P
    xT = np.zeros((n_cores, kf_pad, npc_pad), ml_dtypes.bfloat16)
    xb = x.astype(ml_dtypes.bfloat16)
    for c in range(n_cores):
        nodes_c = np.where(node_core == c)[0]
        xT[c][:nfeat, pos_in_core[nodes_c]] = xb[nodes_c].T
    xT[:, nfeat, :] = 1.0

    meta = dict(npc=npc, nchunks=nchunks, npc_pad=npc_pad, nt=nt, T=T,
                tile_of_chunk=tile_of_chunk, bounds=bounds, rows_q=rows_q,
                hq_base=hq_base, kf_pad=kf_pad,
                node_core=node_core, pos_in_core=pos_in_core)
    return meta, src_dev, S_dev, ST_dev, xT


GT = 6  # tiles per vector group (2 PSUM banks x 3 tiles)


def _group_plan(ntj):
    """Split a chunk's ntj tiles into vector groups over the 3-bank ut
    supertile.  Returns list of (g0, gn, spans, (nfull, rem)) where spans
    are F32-element offsets into the [P,1536] supertile; the Act copy is
    one instruction over nfull full banks plus one for the remainder."""
    plan = []
    g0 = 0
    while g0 < ntj:
        gn = min(GT, ntj - g0)
        nfull, rem = divmod(gn, 3)
        spans = [512 * b + 160 * i for b in range(nfull) for i in range(3)]
        spans += [512 * nfull + 160 * i for i in range(rem)]
        plan.append((g0, gn, spans, (nfull, rem)))
        g0 += gn
    return plan


def build_program(nfeat, d, nclass, meta, n_cores):
    dd = d // K
    npc_pad = meta["npc_pad"]
    nchunks = meta["nchunks"]
    nt = meta["nt"]
    T = meta["T"]
    toc = meta["tile_of_chunk"]
    bounds = meta["bounds"]
    rows_q = meta["rows_q"]
    hq_base = meta["hq_base"]
    kf_pad = meta["kf_pad"]
    nkt = kf_pad // P
    HROWS = int(hq_base[-1])
    max_nt = int(nt.max())

    nc = bass.Bass(num_devices=n_cores)

    xT_t = nc.dram_tensor("xT", [kf_pad, npc_pad], BF16, kind="ExternalInput")
    w_t = nc.dram_tensor("wp", [kf_pad, d], BF16, kind="ExternalInput")
    cw_t = nc.dram_tensor("cwp", [P, 3 * nclass], BF16, kind="ExternalInput")
    src_t = nc.dram_tensor("src", [P, T], I32, kind="ExternalInput")
    S_t = nc.dram_tensor("Smask", [P, T * P], FP8, kind="ExternalInput")
    ST_t = nc.dram_tensor("STmask", [P, T * P], FP8, kind="ExternalInput")
    y_t = nc.dram_tensor("y", [npc_pad, nclass], F32, kind="ExternalOutput")
    Hp = nc.dram_tensor("Hp", [HROWS, d], BF16, kind="Internal")

    with tile.TileContext(nc) as tc:
        with (
            tc.tile_pool(name="persist", bufs=1) as pp,
            tc.tile_pool(name="dram", bufs=1, space="DRAM") as dp,
            tc.tile_pool(name="p0", bufs=2) as sb,
            tc.tile_pool(name="mask", bufs=2) as sm,
            tc.tile_pool(name="zpool", bufs=ZBUFS) as sz,
            tc.tile_pool(name="ring", bufs=2) as sr,
            tc.tile_pool(name="epi", bufs=2) as se,
            tc.tile_pool(name="put", bufs=2, space="PSUM") as put,
            tc.tile_pool(name="pseg", bufs=1, space="PSUM") as pse,
            tc.tile_pool(name="ptr", bufs=1, space="PSUM") as ptr,
        ):
            # ---------------- constants / persistent state ----------------
            ident = pp.tile([P, P], BF16)
            make_identity(nc, ident[:])
            ones_sb = pp.tile([1, P], BF16)
            nc.vector.memset(ones_sb[:], 1.0)
            eps_b = pp.tile([P, 1], F32)
            nc.vector.memset(eps_b[:], 1e-24)

            w_sb = pp.tile([P, nkt * d], BF16)
            nc.sync.dma_start(
                out=w_sb[:].rearrange("p (a q) -> p a q", q=d),
                in_=w_t[:].rearrange("(a p) q -> p a q", p=P),
            )
            cw_sb = pp.tile([P, 3 * nclass], BF16)
            nc.sync.dma_start(out=cw_sb[:], in_=cw_t[:])
            src_sb = pp.tile([P, T], I32)
            nc.sync.dma_start(out=src_sb[:], in_=src_t[:])

            hn = pp.tile([P, nchunks * d], BF16)  # normalized features (own nodes)
            ag_in = dp.tile([npc_pad, d], BF16)

            # ---------------- P0: pca + lrelu + l2norm + sub-allgathers ----
            qnext = 0
            for m in range(nchunks):
                xt = sb.tile([P, nkt * P], BF16, tag="xt", bufs=3)
                nc.sync.dma_start(
                    out=xt[:].rearrange("p (a q) -> p a q", q=P),
                    in_=xT_t[:, m * P:(m + 1) * P].rearrange("(a p) q -> p a q", p=P),
                )
                h_ps = put.tile([P, 1024], F32, space="PSUM", tag="ut")
                for a in range(nkt):
                    nc.tensor.matmul(
                        out=h_ps[:, :d],
                        lhsT=xt[:, a * P:(a + 1) * P],
                        rhs=w_sb[:, a * d:(a + 1) * d],
                        start=(a == 0),
                        stop=(a == nkt - 1),
                    )
                hs = sb.tile([P, d], F32, tag="hs")
                nc.vector.tensor_scalar_mul(out=hs[:], in0=h_ps[:, :d], scalar1=SLOPE)
                h = sb.tile([P, d], F32, tag="h")
                nc.vector.tensor_tensor(out=h[:], in0=h_ps[:, :d], in1=hs[:], op=OP.max)
                sq = sb.tile([P, d], F32, tag="sq")
                nc.scalar.activation(out=sq[:], in_=h[:], func=AF.Square)
                ss = sb.tile([P, K], F32, tag="ss")
                nc.vector.reduce_sum(
                    out=ss[:], in_=sq[:].rearrange("p (k e) -> p k e", k=K),
                    axis=AX.X,
                )
                lg = sb.tile([P, K], F32, tag="lg")
                nc.scalar.activation(out=lg[:], in_=ss[:], func=AF.Ln, bias=eps_b[:, :1])
                rr = sb.tile([P, K], F32, tag="rr")
                nc.scalar.activation(out=rr[:], in_=lg[:], func=AF.Exp, scale=-0.5)
                nc.vector.tensor_tensor(
                    out=hn[:, m * d:(m + 1) * d].rearrange("p (k e) -> p k e", k=K),
                    in0=h[:].rearrange("p (k e) -> p k e", k=K),
                    in1=rr[:].unsqueeze(2).to_broadcast([P, K, dd]),
                    op=OP.mult,
                )
                nc.sync.dma_start(
                    out=ag_in[m * P:(m + 1) * P, :], in_=hn[:, m * d:(m + 1) * d]
                )
                if m == bounds[qnext + 1] - 1:
                    q = qnext
                    nc.gpsimd.collective_compute(
                        "AllGather",
                        OP.bypass,
                        replica_groups=[list(range(n_cores))],
                        ins=[ag_in[bounds[q] * P:bounds[q + 1] * P, :]],
                        outs=[Hp.ap()[int(hq_base[q]):int(hq_base[q + 1]), :]],
                    )
                    qnext += 1

            # ---------------- P3: routing ---------------------------------
            def chunk_prologue(j):
                t0, ntj = int(toc[j]), int(nt[j])
                S_sb = sm.tile([P, max_nt * P], FP8, tag=f"S{j % 3}")
                nc.sync.dma_start(
                    out=S_sb[:, :ntj * P], in_=S_t[:, t0 * P:(t0 + ntj) * P]
                )
                ST_sb = sm.tile([P, max_nt * P], FP8, tag=f"ST{j % 3}")
                nc.sync.dma_start(
                    out=ST_sb[:, :ntj * P], in_=ST_t[:, t0 * P:(t0 + ntj) * P]
                )
                zch = sz.tile([P, max_nt * d], BF16, tag="z")
                for b0 in range(0, ntj, ZBATCH):
                    bn = min(ZBATCH, ntj - b0)
                    nc.gpsimd.indirect_dma_start(
                        out=zch[:, b0 * d:(b0 + bn) * d],
                        out_offset=None,
                        in_=Hp.ap(),
                        in_offset=bass.IndirectOffsetOnAxis(
                            ap=src_sb[:, t0 + b0:t0 + b0 + bn], axis=0
                        ),
                    )
                return dict(j=j, ntj=ntj, zch=zch, S_sb=S_sb, ST_sb=ST_sb,
                            u_j=None, plan=_group_plan(ntj))

            def emit_group(st, it, plan_entry):
                j, zch, S_sb, ST_sb, seg = (
                    st["j"], st["zch"], st["S_sb"], st["ST_sb"], st["seg"])
                hn_j = hn[:, j * d:(j + 1) * d]
                u_rhs = hn_j if it == 0 else st["u_j"][:]
                ti = st["ti"]
                for (g0, gn, spans, (nfull, rem)) in [plan_entry]:
                    utp = put.tile([P, 1024], F32, space="PSUM", tag="ut")
                    for i, t in enumerate(range(g0, g0 + gn)):
                        nc.tensor.matmul(
                            out=utp[:, spans[i]:spans[i] + d],
                            lhsT=ST_sb[:, t * P:(t + 1) * P],
                            rhs=u_rhs,
                            start=True, stop=True,
                        )
                    utb = sr.tile([P, GT * d], BF16, tag="utb")
                    if nfull:
                        nc.scalar.copy(
                            out=utb[:, :nfull * 3 * d],
                            in_=utp[:, :nfull * 512].rearrange(
                                "p (b x) -> p b x", b=nfull)[:, :, :3 * d],
                        )
                    if rem:
                        nc.scalar.copy(
                            out=utb[:, nfull * 3 * d:gn * d],
                            in_=utp[:, nfull * 512:nfull * 512 + rem * d],
                        )
                    zg = zch[:, g0 * d:(g0 + gn) * d]
                    prod = sr.tile([P, GT * d], BF16, tag="prod")
                    nc.vector.tensor_mul(out=prod[:, :gn * d], in0=zg, in1=utb[:, :gn * d])
                    pv = prod[:, :gn * d].rearrange("p (a e) -> p a e", e=dd)
                    t1 = sr.tile([P, GT * d // 2], BF16, tag="t1")
                    nc.vector.tensor_add(
                        out=t1[:, :gn * d // 2].rearrange("p (a e) -> p a e", e=8),
                        in0=pv[:, :, 0:8], in1=pv[:, :, 8:16],
                    )
                    t1v = t1[:, :gn * d // 2].rearrange("p (a e) -> p a e", e=8)
                    t2 = sr.tile([P, GT * d // 4], BF16, tag="t2")
                    nc.vector.tensor_add(
                        out=t2[:, :gn * d // 4].rearrange("p (a e) -> p a e", e=4),
                        in0=t1v[:, :, 0:4], in1=t1v[:, :, 4:8],
                    )
                    t2v = t2[:, :gn * d // 4].rearrange("p (a e) -> p a e", e=4)
                    t3 = sr.tile([P, GT * d // 8], BF16, tag="t3")
                    nc.vector.tensor_add(
                        out=t3[:, :gn * d // 8].rearrange("p (a e) -> p a e", e=2),
                        in0=t2v[:, :, 0:2], in1=t2v[:, :, 2:4],
                    )
                    t3v = t3[:, :gn * d // 8].rearrange("p (a e) -> p a e", e=2)
                    sf = sr.tile([P, GT * K], F32, tag="sf")
                    nc.vector.tensor_add(
                        out=sf[:, :gn * K],
                        in0=t3v[:, :, 0:1].squeeze(2), in1=t3v[:, :, 1:2].squeeze(2),
                    )
                    ef = sr.tile([P, GT * K], BF16, tag="ef")
                    nc.scalar.activation(out=ef[:, :gn * K], in_=sf[:, :gn * K], func=AF.Exp)
                    qf = sr.tile([P, GT], F32, tag="qf")
                    nc.vector.reduce_sum(
                        out=qf[:, :gn],
                        in_=ef[:, :gn * K].rearrange("p (a k) -> p a k", k=K),
                        axis=AX.X,
                    )
                    rf = sr.tile([P, GT], F32, tag="rf")
                    nc.vector.reciprocal(out=rf[:, :gn], in_=qf[:, :gn])
                    pe2 = sr.tile([P, GT * K * 2], BF16, tag="pe2")
                    p2v = pe2[:].rearrange("p (a k two) -> p a k two", k=K, two=2)
                    efv = ef[:, :gn * K].rearrange("p (a k) -> p a k", k=K)
                    rfv = rf[:, :gn].unsqueeze(2).to_broadcast([P, gn, K])
                    nc.vector.tensor_tensor(
                        out=p2v[:, :gn, :, 0:1].squeeze(3), in0=efv, in1=rfv, op=OP.mult)
                    nc.vector.tensor_tensor(
                        out=p2v[:, :gn, :, 1:2].squeeze(3), in0=efv, in1=rfv, op=OP.mult)
                    pex = sr.tile([P, GT * d], BF16, tag="pex")
                    nc.scalar.copy(
                        out=pex.bitcast(F32)[:, :gn * d // 2].rearrange(
                            "p (a e) -> p a e", e=dd // 2),
                        in_=pe2.bitcast(F32)[:, :gn * K].unsqueeze(2).to_broadcast(
                            [P, gn * K, dd // 2]),
                    )
                    msg = sr.tile([P, GT * d], BF16, tag="msg")
                    nc.vector.tensor_mul(out=msg[:, :gn * d], in0=zg, in1=pex[:, :gn * d])
                    for i, t in enumerate(range(g0, g0 + gn)):
                        nc.tensor.matmul(
                            out=seg,
                            lhsT=S_sb[:, t * P:(t + 1) * P],
                            rhs=msg[:, i * d:(i + 1) * d],
                            start=(ti == 0), stop=False,
                        )
                        ti += 1
                st["ti"] = ti

            def chunk_residual(st):
                # + x residual via identity matmul, closes the accumulation
                nc.tensor.matmul(
                    out=st["seg"], lhsT=ident[:],
                    rhs=hn[:, st["j"] * d:(st["j"] + 1) * d],
                    start=False, stop=True)

            def chunk_epilogue(st, it):
                j, seg = st["j"], st["seg"]
                sq2 = se.tile([P, d], F32, tag="sq2")
                nc.scalar.activation(out=sq2[:], in_=seg, func=AF.Square)
                ss2 = se.tile([P, K], F32, tag="ss2")
                nc.vector.reduce_sum(
                    out=ss2[:], in_=sq2[:].rearrange("p (k e) -> p k e", k=K),
                    axis=AX.X,
                )
                lg2 = se.tile([P, K], F32, tag="lg2")
                nc.scalar.activation(out=lg2[:], in_=ss2[:], func=AF.Ln, bias=eps_b[:, :1])
                rr2 = se.tile([P, K], F32, tag="rr2")
                nc.scalar.activation(out=rr2[:], in_=lg2[:], func=AF.Exp, scale=-0.5)
                rrb = rr2[:].unsqueeze(2).to_broadcast([P, K, dd])
                segv = seg.rearrange("p (k e) -> p k e", k=K)
                if it < NITER - 1:
                    u_new = se.tile([P, d], BF16, tag=f"uj{j % 3}", bufs=1)
                    nc.vector.tensor_tensor(
                        out=u_new[:].rearrange("p (k e) -> p k e", k=K),
                        in0=segv, in1=rrb, op=OP.mult)
                    st["u_j"] = u_new
                else:
                    uf = se.tile([P, d], F32, tag="uf")
                    nc.vector.tensor_tensor(
                        out=uf[:].rearrange("p (k e) -> p k e", k=K),
                        in0=segv, in1=rrb, op=OP.mult)
                    us = se.tile([P, d], F32, tag="us")
                    nc.vector.tensor_scalar_mul(out=us[:], in0=uf[:], scalar1=SLOPE)
                    ufb = se.tile([P, d], BF16, tag="ufb")
                    nc.vector.tensor_tensor(out=ufb[:], in0=uf[:], in1=us[:], op=OP.max)
                    trp = ptr.tile([P, 1024], BF16, space="PSUM", tag="tr")
                    nc.tensor.transpose(out=trp[:, :P], in_=ufb[:, :P], identity=ident[:])
                    nc.tensor.transpose(
                        out=trp[:d - P, P:2 * P], in_=ufb[:, P:d], identity=ident[:])
                    uT = se.tile([P, 2 * P], BF16, tag="uT")
                    nc.scalar.copy(out=uT[:, :P], in_=trp[:, :P])
                    nc.scalar.copy(out=uT[:d - P, P:], in_=trp[:d - P, P:2 * P])
                    yp = ptr.tile([P, 1024], BF16, space="PSUM", tag="tr")
                    ypv = yp.bitcast(F32)[:, :nclass]
                    nc.tensor.matmul(
                        out=ypv, lhsT=uT[:, :P], rhs=cw_sb[:, :nclass],
                        start=True, stop=False)
                    nc.tensor.matmul(
                        out=ypv, lhsT=uT[:d - P, P:2 * P],
                        rhs=cw_sb[:d - P, nclass:2 * nclass],
                        start=False, stop=False)
                    nc.tensor.matmul(
                        out=ypv, lhsT=ones_sb[:, :P], rhs=cw_sb[0:1, 2 * nclass:],
                        start=False, stop=True)
                    ysb = se.tile([P, nclass], F32, tag="ysb")
                    nc.scalar.copy(out=ysb[:], in_=ypv)
                    nc.sync.dma_start(out=y_t[j * P:(j + 1) * P, :], in_=ysb[:])

            cgroups = [list(range(j0, min(j0 + 3, nchunks)))
                       for j0 in range(0, nchunks, 3)]
            for cg in cgroups:
                sts = [chunk_prologue(j) for j in cg]
                for it in range(NITER):
                    for st in sts:
                        segt = pse.tile([P, 512], F32, space="PSUM",
                                        tag=f"seg{st['j'] % 3}")
                        st["seg"] = segt[:, 0:d]
                        st["ti"] = 0
                    ng = max(len(st["plan"]) for st in sts)
                    for g in range(ng):
                        for st in sts:
                            if g < len(st["plan"]):
                                emit_group(st, it, st["plan"][g])
                    for st in sts:
                        chunk_residual(st)
                    for st in sts:
                        chunk_epilogue(st, it)
    return nc


_CACHE = {}
TRACE = False
LAST_RESULTS = None


def kernel(x, edge_index, pca_w, pca_b, clf_w, clf_b, n_cores=8, _sim=False):
    x = np.asarray(x, np.float32)
    edge_index = np.asarray(edge_index)
    pca_w = np.asarray(pca_w, np.float32)
    pca_b = np.asarray(pca_b, np.float32)
    clf_w = np.asarray(clf_w, np.float32)
    clf_b = np.asarray(clf_b, np.float32)

    n, nfeat = x.shape
    d = pca_w.shape[1]
    nclass = clf_w.shape[1]

    meta, src_dev, S_dev, ST_dev, xT = _host_prep(x, edge_index, n_cores)

    key = (n, nfeat, d, nclass, tuple(meta["nt"].tolist()))
    if key not in _CACHE:
        _CACHE[key] = build_program(nfeat, d, nclass, meta, n_cores)
        if not _sim:
            _split_multiwaits(_CACHE[key])
    nc = _CACHE[key]

    kf_pad = meta["kf_pad"]
    w_pad = np.zeros((kf_pad, d), ml_dtypes.bfloat16)
    w_pad[:nfeat] = pca_w.astype(ml_dtypes.bfloat16)
    w_pad[nfeat] = pca_b.astype(ml_dtypes.bfloat16)
    cwp = np.zeros((P, 3 * nclass), ml_dtypes.bfloat16)
    cwp[:, :nclass] = clf_w[:P].astype(ml_dtypes.bfloat16)
    cwp[:d - P, nclass:2 * nclass] = clf_w[P:].astype(ml_dtypes.bfloat16)
    cwp[0, 2 * nclass:] = clf_b.astype(ml_dtypes.bfloat16)

    in_maps = []
    for c in range(n_cores):
        in_maps.append({
            "xT": xT[c],
            "wp": w_pad,
            "cwp": cwp,
            "src": src_dev[c],
            "Smask": S_dev[c],
            "STmask": ST_dev[c],
        })

    npc = meta["npc"]
    npc_pad = meta["npc_pad"]
    if _sim:
        from concourse.bass_interp import CoreSim
        assert n_cores == 1
        sim = CoreSim(nc)
        for kk, vv in in_maps[0].items():
            sim.tensor(kk)[:] = vv
        sim.simulate()
        y_dev = np.asarray(sim.tensor("y"))[None]
    else:
        global LAST_RESULTS
        res = run_bass_kernel_spmd(
            nc, in_maps, core_ids=list(range(n_cores)), trace=TRACE
        )
        LAST_RESULTS = res
        y_dev = np.stack([res.results[c]["y"] for c in range(n_cores)], axis=0)

    # un-permute: node nd lives at (core, pos)
    y = np.empty((n, nclass), np.float32)
    y[np.arange(n)] = y_dev[meta["node_core"], meta["pos_in_core"]]
    return y.astype(np.float32)


if __name__ == "__main__":
    import pickle, time
    with open("/tmp/ref_inputs.pkl", "rb") as f:
        inputs = pickle.load(f)
    t0 = time.time()
    y = kernel(**inputs)
    print("kernel() wall time", time.time() - t0)
    np.save("/tmp/kernel_out.npy", y)


# revision 36
# speedup vs baseline: 1.0286x; 1.0157x over previous
"""DisentangledGNN Trainium2 kernel (8 NeuronCores, SPMD) — v2.

Strategy: target-bucketed node sharding (each core owns n/8 nodes and all
edges targeting them), with a host-side degree-balanced node permutation so
every (core, chunk) bucket holds ~equal edge counts.

Per core:
  P0  pca matmul (bf16, bias via ones-row) + leaky_relu + grouped l2norm
      (1/sqrt via exp(-0.5*ln(x)) so the Act engine never switches
      activation tables away from the exp/ln set)
  P1  AllGather of normalized features, split into 4 sub-collectives
      overlapped under P0
  P2  z = Hp[src] edge gather (indirect DMA, batched 4 tiles/instruction)
  P3  3 routing iterations; per 128-edge tile the u[trg] gather and the
      segment-sum scatter are one-hot matmuls whose fp8 mask matrices are
      precomputed on host and streamed via DMA.  Softmax over the 10
      factors: exp on Act, sums/reciprocal on DVE, and the p-broadcast to
      dd=16 via a bf16-pair trick (each p duplicated into a bf16 pair,
      bitcast f32, broadcast x8 on Act = half the elements).
  P4  (last iteration) leaky_relu + classifier matmul, bias via ones-row.
No inter-core communication during routing.
"""

import numpy as np
import ml_dtypes

import concourse.bass as bass
import concourse.mybir as mybir
import concourse.tile as tile
from concourse.masks import make_identity
from concourse.bass_utils import run_bass_kernel_spmd

F32 = mybir.dt.float32
BF16 = mybir.dt.bfloat16
I32 = mybir.dt.int32
FP8 = mybir.dt.float8e4
AF = mybir.ActivationFunctionType
AX = mybir.AxisListType
OP = mybir.AluOpType

K = 10
SLOPE = 0.01
NITER = 3
P = 128
ZBATCH = 1   # tiles per indirect-DMA gather (HW SWDGE only honors [P,1] offsets)
ZBUFS = 20   # chunks of z kept in SBUF (prefetch window)


def _split_multiwaits(nc):
    # This walrus accepts at most 1 sync wait per instruction (2 for
    # EventSemaphore ops); split extras onto preceding same-engine NOPs.
    n = [0]
    for fn in nc.m.functions:
        for blk in fn.blocks:
            newinsts = []
            changed = False
            for ins in blk.instructions:
                si = ins.sync_info
                cap = 2 if "EventSem" in type(ins).__name__ else 1
                if si is not None and len(si.on_wait) > cap:
                    waits = list(si.on_wait)
                    for w in waits[cap:]:
                        n[0] += 1
                        nop = mybir.InstNoOp(name=f"{ins.name}-ws{n[0]}", ins=[], outs=[])
                        nop.engine = ins.engine
                        nop.sync_info = mybir.SyncInfo(on_wait=[w], on_update=[])
                        newinsts.append(nop)
                    si.on_wait = waits[:cap]
                    ins.sync_info = si
                    changed = True
                newinsts.append(ins)
            if changed:
                blk.instructions = newinsts


def _host_prep(x, edge_index, n_cores):
    """Degree-balanced node->(core,chunk,slot) assignment, edge bucketing,
    fp8 one-hot mask matrices, permuted bf16 xT, Hp row mapping."""
    n, nfeat = x.shape
    npc = n // n_cores
    nchunks = (npc + P - 1) // P
    npc_pad = nchunks * P
    src = np.asarray(edge_index[0], np.int64)
    trg = np.asarray(edge_index[1], np.int64)

    deg = np.bincount(trg, minlength=n).astype(np.int64)

    # Greedy: nodes in descending-degree order to the (core,chunk) bin with
    # the fewest edges, subject to <=128 nodes/bin and npc nodes/core.
    order = np.argsort(-deg, kind="stable")
    bin_edges = np.zeros((n_cores, nchunks), np.int64)
    bin_nodes = np.zeros((n_cores, nchunks), np.int64)
    core_nodes = np.zeros(n_cores, np.int64)
    node_core = np.empty(n, np.int32)
    node_chunk = np.empty(n, np.int32)
    node_slot = np.empty(n, np.int32)
    INF = 1 << 60
    for nd in order:
        feas = (bin_nodes < P) & (core_nodes[:, None] < npc)
        masked = np.where(feas, bin_edges, INF)
        ci = int(np.argmin(masked))
        c, j = divmod(ci, nchunks)
        node_core[nd] = c
        node_chunk[nd] = j
        node_slot[nd] = bin_nodes[c, j]
        bin_nodes[c, j] += 1
        core_nodes[c] += 1
        bin_edges[c, j] += deg[nd]

    nt = np.maximum(1, (bin_edges.max(axis=0) + P - 1) // P).astype(np.int64)
    T = int(nt.sum())
    tile_of_chunk = np.concatenate([[0], np.cumsum(nt)]).astype(np.int64)

    # AllGather split points (chunk granularity) and Hp row mapping.
    nsplit = min(2, nchunks)
    bounds = [round(q * nchunks / nsplit) for q in range(nsplit + 1)]
    rows_q = [(bounds[q + 1] - bounds[q]) * P for q in range(nsplit)]
    hq_base = np.concatenate([[0], np.cumsum([n_cores * r for r in rows_q])])
    pos_in_core = node_chunk * P + node_slot
    node_split = np.searchsorted(np.asarray(bounds[1:]) * P, pos_in_core, side="right")
    hp_row = (
        hq_base[node_split]
        + node_core * np.asarray(rows_q)[node_split]
        + (pos_in_core - np.asarray(bounds)[node_split] * P)
    ).astype(np.int32)

    # Edge bucketing per core, chunk-sorted; slots padded with lloc=255.
    e_core = node_core[trg]
    e_chunk = node_chunk[trg]
    e_lloc = node_slot[trg]
    e_srow = hp_row[src]
    eorder = np.lexsort((e_lloc, e_chunk, e_core))
    e_core, e_chunk, e_lloc, e_srow = (
        e_core[eorder], e_chunk[eorder], e_lloc[eorder], e_srow[eorder])

    src_arr = np.zeros((n_cores, T * P), np.int32)
    lloc_arr = np.full((n_cores, T * P), 255, np.int32)
    core_starts = np.searchsorted(e_core, np.arange(n_cores + 1))
    for c in range(n_cores):
        cs, ce = core_starts[c], core_starts[c + 1]
        chunk_c = e_chunk[cs:ce]
        starts = np.searchsorted(chunk_c, np.arange(nchunks + 1))
        for j in range(nchunks):
            e0, e1 = cs + starts[j], cs + starts[j + 1]
            base = int(tile_of_chunk[j]) * P
            cnt = e1 - e0
            src_arr[c, base:base + cnt] = e_srow[e0:e1]
            lloc_arr[c, base:base + cnt] = e_lloc[e0:e1]

    # Device layouts: slot s -> tile s//P, lane s%P  => [P, T]
    src_dev = src_arr.reshape(n_cores, T, P).transpose(0, 2, 1).copy()
    lloc_mat = lloc_arr.reshape(n_cores, T, P).transpose(0, 2, 1)  # [c, P, T]

    # fp8 one-hot masks.  S[e-lane, t, v] = (lloc==v); ST is per-tile transpose.
    ar = np.arange(P)
    S_bool = lloc_mat[:, :, :, None] == ar[None, None, None, :]     # [c,P,T,128]
    ST_bool = S_bool.transpose(0, 3, 2, 1)                          # [c,P,T,128]
    S_dev = S_bool.astype(ml_dtypes.float8_e4m3fn).reshape(n_cores, P, T * P)
    ST_dev = np.ascontiguousarray(ST_bool).astype(ml_dtypes.float8_e4m3fn).reshape(n_cores, P, T * P)

    # Permuted xT in bf16, ones row for the pca bias.
    kf_pad = ((nfeat + 1 + P - 1) // P) * P
    xT = np.zeros((n_cores, kf_pad, npc_pad), ml_dtypes.bfloat16)
    xb = x.astype(ml_dtypes.bfloat16)
    for c in range(n_cores):
        nodes_c = np.where(node_core == c)[0]
        xT[c][:nfeat, pos_in_core[nodes_c]] = xb[nodes_c].T
    xT[:, nfeat, :] = 1.0

    meta = dict(npc=npc, nchunks=nchunks, npc_pad=npc_pad, nt=nt, T=T,
                tile_of_chunk=tile_of_chunk, bounds=bounds, rows_q=rows_q,
                hq_base=hq_base, kf_pad=kf_pad,
                node_core=node_core, pos_in_core=pos_in_core)
    return meta, src_dev, S_dev, ST_dev, xT


GT = 6  # tiles per vector group (2 PSUM banks x 3 tiles)


def _group_plan(ntj):
    """Split a chunk's ntj tiles into vector groups over the 3-bank ut
    supertile.  Returns list of (g0, gn, spans, (nfull, rem)) where spans
    are F32-element offsets into the [P,1536] supertile; the Act copy is
    one instruction over nfull full banks plus one for the remainder."""
    plan = []
    g0 = 0
    while g0 < ntj:
        gn = min(GT, ntj - g0)
        nfull, rem = divmod(gn, 3)
        spans = [512 * b + 160 * i for b in range(nfull) for i in range(3)]
        spans += [512 * nfull + 160 * i for i in range(rem)]
        plan.append((g0, gn, spans, (nfull, rem)))
        g0 += gn
    return plan


def build_program(nfeat, d, nclass, meta, n_cores):
    dd = d // K
    npc_pad = meta["npc_pad"]
    nchunks = meta["nchunks"]
    nt = meta["nt"]
    T = meta["T"]
    toc = meta["tile_of_chunk"]
    bounds = meta["bounds"]
    rows_q = meta["rows_q"]
    hq_base = meta["hq_base"]
    kf_pad = meta["kf_pad"]
    nkt = kf_pad // P
    HROWS = int(hq_base[-1])
    max_nt = int(nt.max())

    nc = bass.Bass(num_devices=n_cores)

    xT_t = nc.dram_tensor("xT", [kf_pad, npc_pad], BF16, kind="ExternalInput")
    w_t = nc.dram_tensor("wp", [kf_pad, d], BF16, kind="ExternalInput")
    cw_t = nc.dram_tensor("cwp", [P, 3 * nclass], BF16, kind="ExternalInput")
    src_t = nc.dram_tensor("src", [P, T], I32, kind="ExternalInput")
    S_t = nc.dram_tensor("Smask", [P, T * P], FP8, kind="ExternalInput")
    ST_t = nc.dram_tensor("STmask", [P, T * P], FP8, kind="ExternalInput")
    y_t = nc.dram_tensor("y", [npc_pad, nclass], F32, kind="ExternalOutput")
    Hp = nc.dram_tensor("Hp", [HROWS, d], BF16, kind="Internal")

    with tile.TileContext(nc) as tc:
        with (
            tc.tile_pool(name="persist", bufs=1) as pp,
            tc.tile_pool(name="dram", bufs=1, space="DRAM") as dp,
            tc.tile_pool(name="p0", bufs=2) as sb,
            tc.tile_pool(name="mask", bufs=2) as sm,
            tc.tile_pool(name="zpool", bufs=ZBUFS) as sz,
            tc.tile_pool(name="ring", bufs=2) as sr,
            tc.tile_pool(name="epi", bufs=2) as se,
            tc.tile_pool(name="put", bufs=2, space="PSUM") as put,
            tc.tile_pool(name="pseg", bufs=1, space="PSUM") as pse,
            tc.tile_pool(name="ptr", bufs=1, space="PSUM") as ptr,
        ):
            # ---------------- constants / persistent state ----------------
            ident = pp.tile([P, P], BF16)
            make_identity(nc, ident[:])
            ones_sb = pp.tile([1, P], BF16)
            nc.vector.memset(ones_sb[:], 1.0)
            eps_b = pp.tile([P, 1], F32)
            nc.vector.memset(eps_b[:], 1e-24)

            w_sb = pp.tile([P, nkt * d], BF16)
            nc.sync.dma_start(
                out=w_sb[:].rearrange("p (a q) -> p a q", q=d),
                in_=w_t[:].rearrange("(a p) q -> p a q", p=P),
            )
            cw_sb = pp.tile([P, 3 * nclass], BF16)
            nc.sync.dma_start(out=cw_sb[:], in_=cw_t[:])
            src_sb = pp.tile([P, T], I32)
            nc.sync.dma_start(out=src_sb[:], in_=src_t[:])

            hn = pp.tile([P, nchunks * d], BF16)  # normalized features (own nodes)
            ag_in = dp.tile([npc_pad, d], BF16)

            # ---------------- P0: pca + lrelu + l2norm + sub-allgathers ----
            qnext = 0
            for m in range(nchunks):
                xt = sb.tile([P, nkt * P], BF16, tag="xt", bufs=3)
                nc.sync.dma_start(
                    out=xt[:].rearrange("p (a q) -> p a q", q=P),
                    in_=xT_t[:, m * P:(m + 1) * P].rearrange("(a p) q -> p a q", p=P),
                )
                h_ps = put.tile([P, 1024], F32, space="PSUM", tag="ut")
                for a in range(nkt):
                    nc.tensor.matmul(
                        out=h_ps[:, :d],
                        lhsT=xt[:, a * P:(a + 1) * P],
                        rhs=w_sb[:, a * d:(a + 1) * d],
                        start=(a == 0),
                        stop=(a == nkt - 1),
                    )
                hs = sb.tile([P, d], F32, tag="hs")
                nc.vector.tensor_scalar_mul(out=hs[:], in0=h_ps[:, :d], scalar1=SLOPE)
                h = sb.tile([P, d], F32, tag="h")
                nc.vector.tensor_tensor(out=h[:], in0=h_ps[:, :d], in1=hs[:], op=OP.max)
                sq = sb.tile([P, d], F32, tag="sq")
                nc.scalar.activation(out=sq[:], in_=h[:], func=AF.Square)
                ss = sb.tile([P, K], F32, tag="ss")
                nc.vector.reduce_sum(
                    out=ss[:], in_=sq[:].rearrange("p (k e) -> p k e", k=K),
                    axis=AX.X,
                )
                lg = sb.tile([P, K], F32, tag="lg")
                nc.scalar.activation(out=lg[:], in_=ss[:], func=AF.Ln, bias=eps_b[:, :1])
                rr = sb.tile([P, K], F32, tag="rr")
                nc.scalar.activation(out=rr[:], in_=lg[:], func=AF.Exp, scale=-0.5)
                nc.vector.tensor_tensor(
                    out=hn[:, m * d:(m + 1) * d].rearrange("p (k e) -> p k e", k=K),
                    in0=h[:].rearrange("p (k e) -> p k e", k=K),
                    in1=rr[:].unsqueeze(2).to_broadcast([P, K, dd]),
                    op=OP.mult,
                )
                nc.sync.dma_start(
                    out=ag_in[m * P:(m + 1) * P, :], in_=hn[:, m * d:(m + 1) * d]
                )
                if m == bounds[qnext + 1] - 1:
                    q = qnext
                    nc.gpsimd.collective_compute(
                        "AllGather",
                        OP.bypass,
                        replica_groups=[list(range(n_cores))],
                        ins=[ag_in[bounds[q] * P:bounds[q + 1] * P, :]],
                        outs=[Hp.ap()[int(hq_base[q]):int(hq_base[q + 1]), :]],
                    )
                    qnext += 1

            # ---------------- P3: routing ---------------------------------
            def chunk_prologue(j):
                t0, ntj = int(toc[j]), int(nt[j])
                S_sb = sm.tile([P, max_nt * P], FP8, tag=f"S{j % 3}")
                nc.sync.dma_start(
                    out=S_sb[:, :ntj * P], in_=S_t[:, t0 * P:(t0 + ntj) * P]
                )
                ST_sb = sm.tile([P, max_nt * P], FP8, tag=f"ST{j % 3}")
                nc.sync.dma_start(
                    out=ST_sb[:, :ntj * P], in_=ST_t[:, t0 * P:(t0 + ntj) * P]
                )
                zch = sz.tile([P, max_nt * d], BF16, tag="z")
                for b0 in range(0, ntj, ZBATCH):
                    bn = min(ZBATCH, ntj - b0)
                    nc.gpsimd.indirect_dma_start(
                        out=zch[:, b0 * d:(b0 + bn) * d],
                        out_offset=None,
                        in_=Hp.ap(),
                        in_offset=bass.IndirectOffsetOnAxis(
                            ap=src_sb[:, t0 + b0:t0 + b0 + bn], axis=0
                        ),
                    )
                return dict(j=j, ntj=ntj, zch=zch, S_sb=S_sb, ST_sb=ST_sb,
                            u_j=None, plan=_group_plan(ntj))

            def emit_group(st, it, plan_entry):
                j, zch, S_sb, ST_sb, seg = (
                    st["j"], st["zch"], st["S_sb"], st["ST_sb"], st["seg"])
                hn_j = hn[:, j * d:(j + 1) * d]
                u_rhs = hn_j if it == 0 else st["u_j"][:]
                ti = st["ti"]
                for (g0, gn, spans, (nfull, rem)) in [plan_entry]:
                    utp = put.tile([P, 1024], F32, space="PSUM", tag="ut")
                    for i, t in enumerate(range(g0, g0 + gn)):
                        nc.tensor.matmul(
                            out=utp[:, spans[i]:spans[i] + d],
                            lhsT=ST_sb[:, t * P:(t + 1) * P],
                            rhs=u_rhs,
                            start=True, stop=True,
                        )
                    utb = sr.tile([P, GT * d], BF16, tag="utb")
                    if nfull:
                        nc.scalar.copy(
                            out=utb[:, :nfull * 3 * d],
                            in_=utp[:, :nfull * 512].rearrange(
                                "p (b x) -> p b x", b=nfull)[:, :, :3 * d],
                        )
                    if rem:
                        nc.scalar.copy(
                            out=utb[:, nfull * 3 * d:gn * d],
                            in_=utp[:, nfull * 512:nfull * 512 + rem * d],
                        )
                    zg = zch[:, g0 * d:(g0 + gn) * d]
                    prod = sr.tile([P, GT * d], BF16, tag="prod")
                    nc.vector.tensor_mul(out=prod[:, :gn * d], in0=zg, in1=utb[:, :gn * d])
                    pv = prod[:, :gn * d].rearrange("p (a e) -> p a e", e=dd)
                    t1 = sr.tile([P, GT * d // 2], BF16, tag="t1")
                    nc.vector.tensor_add(
                        out=t1[:, :gn * d // 2].rearrange("p (a e) -> p a e", e=8),
                        in0=pv[:, :, 0:8], in1=pv[:, :, 8:16],
                    )
                    t1v = t1[:, :gn * d // 2].rearrange("p (a e) -> p a e", e=8)
                    t2 = sr.tile([P, GT * d // 4], BF16, tag="t2")
                    nc.vector.tensor_add(
                        out=t2[:, :gn * d // 4].rearrange("p (a e) -> p a e", e=4),
                        in0=t1v[:, :, 0:4], in1=t1v[:, :, 4:8],
                    )
                    t2v = t2[:, :gn * d // 4].rearrange("p (a e) -> p a e", e=4)
                    t3 = sr.tile([P, GT * d // 8], BF16, tag="t3")
                    nc.vector.tensor_add(
                        out=t3[:, :gn * d // 8].rearrange("p (a e) -> p a e", e=2),
                        in0=t2v[:, :, 0:2], in1=t2v[:, :, 2:4],
                    )
                    t3v = t3[:, :gn * d // 8].rearrange("p (a e) -> p a e", e=2)
                    sf = sr.tile([P, GT * K], F32, tag="sf")
                    nc.vector.tensor_add(
                        out=sf[:, :gn * K],
                        in0=t3v[:, :, 0:1].squeeze(2), in1=t3v[:, :, 1:2].squeeze(2),
                    )
                    ef = sr.tile([P, GT * K], BF16, tag="ef")
                    nc.scalar.activation(out=ef[:, :gn * K], in_=sf[:, :gn * K], func=AF.Exp)
                    qf = sr.tile([P, GT], F32, tag="qf")
                    nc.vector.reduce_sum(
                        out=qf[:, :gn],
                        in_=ef[:, :gn * K].rearrange("p (a k) -> p a k", k=K),
                        axis=AX.X,
                    )
                    rf = sr.tile([P, GT], F32, tag="rf")
                    nc.vector.reciprocal(out=rf[:, :gn], in_=qf[:, :gn])
                    pe2 = sr.tile([P, GT * K * 2], BF16, tag="pe2")
                    p2v = pe2[:].rearrange("p (a k two) -> p a k two", k=K, two=2)
                    efv = ef[:, :gn * K].rearrange("p (a k) -> p a k", k=K)
                    rfv = rf[:, :gn].unsqueeze(2).to_broadcast([P, gn, K])
                    nc.vector.tensor_tensor(
                        out=p2v[:, :gn, :, 0:1].squeeze(3), in0=efv, in1=rfv, op=OP.mult)
                    nc.vector.tensor_tensor(
                        out=p2v[:, :gn, :, 1:2].squeeze(3), in0=efv, in1=rfv, op=OP.mult)
                    pex = sr.tile([P, GT * d], BF16, tag="pex")
                    nc.scalar.copy(
                        out=pex.bitcast(F32)[:, :gn * d // 2].rearrange(
                            "p (a e) -> p a e", e=dd // 2),
                        in_=pe2.bitcast(F32)[:, :gn * K].unsqueeze(2).to_broadcast(
                            [P, gn * K, dd // 2]),
                    )
                    msg = sr.tile([P, GT * d], BF16, tag="msg")
                    nc.vector.tensor_mul(out=msg[:, :gn * d], in0=zg, in1=pex[:, :gn * d])
                    for i, t in enumerate(range(g0, g0 + gn)):
                        nc.tensor.matmul(
                            out=seg,
                            lhsT=S_sb[:, t * P:(t + 1) * P],
                            rhs=msg[:, i * d:(i + 1) * d],
                            start=(ti == 0), stop=False,
                        )
                        ti += 1
                st["ti"] = ti

            def chunk_residual(st):
                # + x residual via identity matmul, closes the accumulation
                nc.tensor.matmul(
                    out=st["seg"], lhsT=ident[:],
                    rhs=hn[:, st["j"] * d:(st["j"] + 1) * d],
                    start=False, stop=True)

            def chunk_epilogue(st, it):
                j, seg = st["j"], st["seg"]
                sq2 = se.tile([P, d], F32, tag="sq2")
                nc.scalar.activation(out=sq2[:], in_=seg, func=AF.Square)
                ss2 = se.tile([P, K], F32, tag="ss2")
                nc.vector.reduce_sum(
                    out=ss2[:], in_=sq2[:].rearrange("p (k e) -> p k e", k=K),
                    axis=AX.X,
                )
                lg2 = se.tile([P, K], F32, tag="lg2")
                nc.scalar.activation(out=lg2[:], in_=ss2[:], func=AF.Ln, bias=eps_b[:, :1])
                rr2 = se.tile([P, K], F32, tag="rr2")
                nc.scalar.activation(out=rr2[:], in_=lg2[:], func=AF.Exp, scale=-0.5)
                rrb = rr2[:].unsqueeze(2).to_broadcast([P, K, dd])
                segv = seg.rearrange("p (k e) -> p k e", k=K)
                if it < NITER - 1:
                    u_new = se.tile([P, d], BF16, tag=f"uj{j % 3}", bufs=1)
                    nc.vector.tensor_tensor(
                        out=u_new[:].rearrange("p (k e) -> p k e", k=K),
                        in0=segv, in1=rrb, op=OP.mult)
                    st["u_j"] = u_new
                else:
                    uf = se.tile([P, d], F32, tag="uf")
                    nc.vector.tensor_tensor(
                        out=uf[:].rearrange("p (k e) -> p k e", k=K),
                        in0=segv, in1=rrb, op=OP.mult)
                    us = se.tile([P, d], F32, tag="us")
                    nc.vector.tensor_scalar_mul(out=us[:], in0=uf[:], scalar1=SLOPE)
                    ufb = se.tile([P, d], BF16, tag="ufb")
                    nc.vector.tensor_tensor(out=ufb[:], in0=uf[:], in1=us[:], op=OP.max)
                    trp = ptr.tile([P, 1024], BF16, space="PSUM", tag="tr")
                    nc.tensor.transpose(out=trp[:, :P], in_=ufb[:, :P], identity=ident[:])
                    nc.tensor.transpose(
                        out=trp[:d - P, P:2 * P], in_=ufb[:, P:d], identity=ident[:])
                    uT = se.tile([P, 2 * P], BF16, tag="uT")
                    nc.scalar.copy(out=uT[:, :P], in_=trp[:, :P])
                    nc.scalar.copy(out=uT[:d - P, P:], in_=trp[:d - P, P:2 * P])
                    yp = ptr.tile([P, 1024], BF16, space="PSUM", tag="tr")
                    ypv = yp.bitcast(F32)[:, :nclass]
                    nc.tensor.matmul(
                        out=ypv, lhsT=uT[:, :P], rhs=cw_sb[:, :nclass],
                        start=True, stop=False)
                    nc.tensor.matmul(
                        out=ypv, lhsT=uT[:d - P, P:2 * P],
                        rhs=cw_sb[:d - P, nclass:2 * nclass],
                        start=False, stop=False)
                    nc.tensor.matmul(
                        out=ypv, lhsT=ones_sb[:, :P], rhs=cw_sb[0:1, 2 * nclass:],
                        start=False, stop=True)
                    ysb = se.tile([P, nclass], F32, tag="ysb")
                    nc.scalar.copy(out=ysb[:], in_=ypv)
                    nc.sync.dma_start(out=y_t[j * P:(j + 1) * P, :], in_=ysb[:])

            # triples, but finish with two pairs so the pipeline drain after
            # the last z-gather is short
            if nchunks > 7 and nchunks % 3 == 1:
                ntrip = (nchunks - 4) // 3
                cgroups = [list(range(j0, j0 + 3))
                           for j0 in range(0, 3 * ntrip, 3)]
                cgroups += [[nchunks - 4, nchunks - 3], [nchunks - 2, nchunks - 1]]
            else:
                cgroups = [list(range(j0, min(j0 + 3, nchunks)))
                           for j0 in range(0, nchunks, 3)]
            for cg in cgroups:
                sts = [chunk_prologue(j) for j in cg]
                for it in range(NITER):
                    for st in sts:
                        segt = pse.tile([P, 512], F32, space="PSUM",
                                        tag=f"seg{st['j'] % 3}")
                        st["seg"] = segt[:, 0:d]
                        st["ti"] = 0
                    ng = max(len(st["plan"]) for st in sts)
                    for g in range(ng):
                        for st in sts:
                            if g < len(st["plan"]):
                                emit_group(st, it, st["plan"][g])
                    for st in sts:
                        chunk_residual(st)
                    for st in sts:
                        chunk_epilogue(st, it)
    return nc


_CACHE = {}
TRACE = False
LAST_RESULTS = None


def kernel(x, edge_index, pca_w, pca_b, clf_w, clf_b, n_cores=8, _sim=False):
    x = np.asarray(x, np.float32)
    edge_index = np.asarray(edge_index)
    pca_w = np.asarray(pca_w, np.float32)
    pca_b = np.asarray(pca_b, np.float32)
    clf_w = np.asarray(clf_w, np.float32)
    clf_b = np.asarray(clf_b, np.float32)

    n, nfeat = x.shape
    d = pca_w.shape[1]
    nclass = clf_w.shape[1]

    meta, src_dev, S_dev, ST_dev, xT = _host_prep(x, edge_index, n_cores)

    key = (n, nfeat, d, nclass, tuple(meta["nt"].tolist()))
    if key not in _CACHE:
        _CACHE[key] = build_program(nfeat, d, nclass, meta, n_cores)
        if not _sim:
            _split_multiwaits(_CACHE[key])
    nc = _CACHE[key]

    kf_pad = meta["kf_pad"]
    w_pad = np.zeros((kf_pad, d), ml_dtypes.bfloat16)
    w_pad[:nfeat] = pca_w.astype(ml_dtypes.bfloat16)
    w_pad[nfeat] = pca_b.astype(ml_dtypes.bfloat16)
    cwp = np.zeros((P, 3 * nclass), ml_dtypes.bfloat16)
    cwp[:, :nclass] = clf_w[:P].astype(ml_dtypes.bfloat16)
    cwp[:d - P, nclass:2 * nclass] = clf_w[P:].astype(ml_dtypes.bfloat16)
    cwp[0, 2 * nclass:] = clf_b.astype(ml_dtypes.bfloat16)

    in_maps = []
    for c in range(n_cores):
        in_maps.append({
            "xT": xT[c],
            "wp": w_pad,
            "cwp": cwp,
            "src": src_dev[c],
            "Smask": S_dev[c],
            "STmask": ST_dev[c],
        })

    npc = meta["npc"]
    npc_pad = meta["npc_pad"]
    if _sim:
        from concourse.bass_interp import CoreSim
        assert n_cores == 1
        sim = CoreSim(nc)
        for kk, vv in in_maps[0].items():
            sim.tensor(kk)[:] = vv
        sim.simulate()
        y_dev = np.asarray(sim.tensor("y"))[None]
    else:
        global LAST_RESULTS
        res = run_bass_kernel_spmd(
            nc, in_maps, core_ids=list(range(n_cores)), trace=TRACE
        )
        LAST_RESULTS = res
        y_dev = np.stack([res.results[c]["y"] for c in range(n_cores)], axis=0)

    # un-permute: node nd lives at (core, pos)
    y = np.empty((n, nclass), np.float32)
    y[np.arange(n)] = y_dev[meta["node_core"], meta["pos_in_core"]]
    return y.astype(np.float32)


if __name__ == "__main__":
    import pickle, time
    with open("/tmp/ref_inputs.pkl", "rb") as f:
        inputs = pickle.load(f)
    t0 = time.time()
    y = kernel(**inputs)
    print("kernel() wall time", time.time() - t0)
    np.save("/tmp/kernel_out.npy", y)


# revision 37
# speedup vs baseline: 1.0313x; 1.0027x over previous
"""DisentangledGNN Trainium2 kernel (8 NeuronCores, SPMD) — v2.

Strategy: target-bucketed node sharding (each core owns n/8 nodes and all
edges targeting them), with a host-side degree-balanced node permutation so
every (core, chunk) bucket holds ~equal edge counts.

Per core:
  P0  pca matmul (bf16, bias via ones-row) + leaky_relu + grouped l2norm
      (1/sqrt via exp(-0.5*ln(x)) so the Act engine never switches
      activation tables away from the exp/ln set)
  P1  AllGather of normalized features, split into 4 sub-collectives
      overlapped under P0
  P2  z = Hp[src] edge gather (indirect DMA, batched 4 tiles/instruction)
  P3  3 routing iterations; per 128-edge tile the u[trg] gather and the
      segment-sum scatter are one-hot matmuls whose fp8 mask matrices are
      precomputed on host and streamed via DMA.  Softmax over the 10
      factors: exp on Act, sums/reciprocal on DVE, and the p-broadcast to
      dd=16 via a bf16-pair trick (each p duplicated into a bf16 pair,
      bitcast f32, broadcast x8 on Act = half the elements).
  P4  (last iteration) leaky_relu + classifier matmul, bias via ones-row.
No inter-core communication during routing.
"""

import numpy as np
import ml_dtypes

import concourse.bass as bass
import concourse.mybir as mybir
import concourse.tile as tile
from concourse.masks import make_identity
from concourse.bass_utils import run_bass_kernel_spmd

F32 = mybir.dt.float32
BF16 = mybir.dt.bfloat16
I32 = mybir.dt.int32
FP8 = mybir.dt.float8e4
AF = mybir.ActivationFunctionType
AX = mybir.AxisListType
OP = mybir.AluOpType

K = 10
SLOPE = 0.01
NITER = 3
P = 128
ZBATCH = 1   # tiles per indirect-DMA gather (HW SWDGE only honors [P,1] offsets)
ZBUFS = 20   # chunks of z kept in SBUF (prefetch window)


def _split_multiwaits(nc):
    # This walrus accepts at most 1 sync wait per instruction (2 for
    # EventSemaphore ops); split extras onto preceding same-engine NOPs.
    n = [0]
    for fn in nc.m.functions:
        for blk in fn.blocks:
            newinsts = []
            changed = False
            for ins in blk.instructions:
                si = ins.sync_info
                cap = 2 if "EventSem" in type(ins).__name__ else 1
                if si is not None and len(si.on_wait) > cap:
                    waits = list(si.on_wait)
                    for w in waits[cap:]:
                        n[0] += 1
                        nop = mybir.InstNoOp(name=f"{ins.name}-ws{n[0]}", ins=[], outs=[])
                        nop.engine = ins.engine
                        nop.sync_info = mybir.SyncInfo(on_wait=[w], on_update=[])
                        newinsts.append(nop)
                    si.on_wait = waits[:cap]
                    ins.sync_info = si
                    changed = True
                newinsts.append(ins)
            if changed:
                blk.instructions = newinsts


def _host_prep(x, edge_index, n_cores):
    """Degree-balanced node->(core,chunk,slot) assignment, edge bucketing,
    fp8 one-hot mask matrices, permuted bf16 xT, Hp row mapping."""
    n, nfeat = x.shape
    npc = n // n_cores
    nchunks = (npc + P - 1) // P
    npc_pad = nchunks * P
    src = np.asarray(edge_index[0], np.int64)
    trg = np.asarray(edge_index[1], np.int64)

    deg = np.bincount(trg, minlength=n).astype(np.int64)

    # Greedy: nodes in descending-degree order to the (core,chunk) bin with
    # the fewest edges, subject to <=128 nodes/bin and npc nodes/core.
    order = np.argsort(-deg, kind="stable")
    bin_edges = np.zeros((n_cores, nchunks), np.int64)
    bin_nodes = np.zeros((n_cores, nchunks), np.int64)
    core_nodes = np.zeros(n_cores, np.int64)
    node_core = np.empty(n, np.int32)
    node_chunk = np.empty(n, np.int32)
    node_slot = np.empty(n, np.int32)
    INF = 1 << 60
    for nd in order:
        feas = (bin_nodes < P) & (core_nodes[:, None] < npc)
        masked = np.where(feas, bin_edges, INF)
        ci = int(np.argmin(masked))
        c, j = divmod(ci, nchunks)
        node_core[nd] = c
        node_chunk[nd] = j
        node_slot[nd] = bin_nodes[c, j]
        bin_nodes[c, j] += 1
        core_nodes[c] += 1
        bin_edges[c, j] += deg[nd]

    nt = np.maximum(1, (bin_edges.max(axis=0) + P - 1) // P).astype(np.int64)
    T = int(nt.sum())
    tile_of_chunk = np.concatenate([[0], np.cumsum(nt)]).astype(np.int64)

    # AllGather split points (chunk granularity) and Hp row mapping.
    nsplit = min(2, nchunks)
    bounds = [round(q * nchunks / nsplit) for q in range(nsplit + 1)]
    rows_q = [(bounds[q + 1] - bounds[q]) * P for q in range(nsplit)]
    hq_base = np.concatenate([[0], np.cumsum([n_cores * r for r in rows_q])])
    pos_in_core = node_chunk * P + node_slot
    node_split = np.searchsorted(np.asarray(bounds[1:]) * P, pos_in_core, side="right")
    hp_row = (
        hq_base[node_split]
        + node_core * np.asarray(rows_q)[node_split]
        + (pos_in_core - np.asarray(bounds)[node_split] * P)
    ).astype(np.int32)

    # Edge bucketing per core, chunk-sorted; slots padded with lloc=255.
    e_core = node_core[trg]
    e_chunk = node_chunk[trg]
    e_lloc = node_slot[trg]
    e_srow = hp_row[src]
    eorder = np.lexsort((e_lloc, e_chunk, e_core))
    e_core, e_chunk, e_lloc, e_srow = (
        e_core[eorder], e_chunk[eorder], e_lloc[eorder], e_srow[eorder])

    src_arr = np.zeros((n_cores, T * P), np.int32)
    lloc_arr = np.full((n_cores, T * P), 255, np.int32)
    core_starts = np.searchsorted(e_core, np.arange(n_cores + 1))
    for c in range(n_cores):
        cs, ce = core_starts[c], core_starts[c + 1]
        chunk_c = e_chunk[cs:ce]
        starts = np.searchsorted(chunk_c, np.arange(nchunks + 1))
        for j in range(nchunks):
            e0, e1 = cs + starts[j], cs + starts[j + 1]
            base = int(tile_of_chunk[j]) * P
            cnt = e1 - e0
            src_arr[c, base:base + cnt] = e_srow[e0:e1]
            lloc_arr[c, base:base + cnt] = e_lloc[e0:e1]

    # Device layouts: slot s -> tile s//P, lane s%P  => [P, T]
    src_dev = src_arr.reshape(n_cores, T, P).transpose(0, 2, 1).copy()
    lloc_mat = lloc_arr.reshape(n_cores, T, P).transpose(0, 2, 1)  # [c, P, T]

    # fp8 one-hot masks.  S[e-lane, t, v] = (lloc==v); ST is per-tile transpose.
    ar = np.arange(P)
    S_bool = lloc_mat[:, :, :, None] == ar[None, None, None, :]     # [c,P,T,128]
    ST_bool = S_bool.transpose(0, 3, 2, 1)                          # [c,P,T,128]
    S_dev = S_bool.astype(ml_dtypes.float8_e4m3fn).reshape(n_cores, P, T * P)
    ST_dev = np.ascontiguousarray(ST_bool).astype(ml_dtypes.float8_e4m3fn).reshape(n_cores, P, T * P)

    # Permuted xT in bf16, ones row for the pca bias.
    kf_pad = ((nfeat + 1 + P - 1) // P) * P
    xT = np.zeros((n_cores, kf_pad, npc_pad), ml_dtypes.bfloat16)
    xb = x.astype(ml_dtypes.bfloat16)
    for c in range(n_cores):
        nodes_c = np.where(node_core == c)[0]
        xT[c][:nfeat, pos_in_core[nodes_c]] = xb[nodes_c].T
    xT[:, nfeat, :] = 1.0

    meta = dict(npc=npc, nchunks=nchunks, npc_pad=npc_pad, nt=nt, T=T,
                tile_of_chunk=tile_of_chunk, bounds=bounds, rows_q=rows_q,
                hq_base=hq_base, kf_pad=kf_pad,
                node_core=node_core, pos_in_core=pos_in_core)
    return meta, src_dev, S_dev, ST_dev, xT


GT = 6  # tiles per vector group (2 PSUM banks x 3 tiles)


def _group_plan(ntj):
    """Split a chunk's ntj tiles into vector groups over the 3-bank ut
    supertile.  Returns list of (g0, gn, spans, (nfull, rem)) where spans
    are F32-element offsets into the [P,1536] supertile; the Act copy is
    one instruction over nfull full banks plus one for the remainder."""
    plan = []
    g0 = 0
    while g0 < ntj:
        gn = min(GT, ntj - g0)
        nfull, rem = divmod(gn, 3)
        spans = [512 * b + 160 * i for b in range(nfull) for i in range(3)]
        spans += [512 * nfull + 160 * i for i in range(rem)]
        plan.append((g0, gn, spans, (nfull, rem)))
        g0 += gn
    return plan


def build_program(nfeat, d, nclass, meta, n_cores):
    dd = d // K
    npc_pad = meta["npc_pad"]
    nchunks = meta["nchunks"]
    nt = meta["nt"]
    T = meta["T"]
    toc = meta["tile_of_chunk"]
    bounds = meta["bounds"]
    rows_q = meta["rows_q"]
    hq_base = meta["hq_base"]
    kf_pad = meta["kf_pad"]
    nkt = kf_pad // P
    HROWS = int(hq_base[-1])
    max_nt = int(nt.max())

    nc = bass.Bass(num_devices=n_cores)

    xT_t = nc.dram_tensor("xT", [kf_pad, npc_pad], BF16, kind="ExternalInput")
    w_t = nc.dram_tensor("wp", [kf_pad, d], BF16, kind="ExternalInput")
    cw_t = nc.dram_tensor("cwp", [P, 3 * nclass], BF16, kind="ExternalInput")
    src_t = nc.dram_tensor("src", [P, T], I32, kind="ExternalInput")
    S_t = nc.dram_tensor("Smask", [P, T * P], FP8, kind="ExternalInput")
    ST_t = nc.dram_tensor("STmask", [P, T * P], FP8, kind="ExternalInput")
    y_t = nc.dram_tensor("y", [npc_pad, nclass], F32, kind="ExternalOutput")
    Hp = nc.dram_tensor("Hp", [HROWS, d], BF16, kind="Internal")

    with tile.TileContext(nc) as tc:
        with (
            tc.tile_pool(name="persist", bufs=1) as pp,
            tc.tile_pool(name="dram", bufs=1, space="DRAM") as dp,
            tc.tile_pool(name="p0", bufs=2) as sb,
            tc.tile_pool(name="mask", bufs=2) as sm,
            tc.tile_pool(name="zpool", bufs=ZBUFS) as sz,
            tc.tile_pool(name="ring", bufs=3) as sr,
            tc.tile_pool(name="epi", bufs=3) as se,
            tc.tile_pool(name="put", bufs=2, space="PSUM") as put,
            tc.tile_pool(name="pseg", bufs=1, space="PSUM") as pse,
            tc.tile_pool(name="ptr", bufs=1, space="PSUM") as ptr,
        ):
            # ---------------- constants / persistent state ----------------
            ident = pp.tile([P, P], BF16)
            make_identity(nc, ident[:])
            ones_sb = pp.tile([1, P], BF16)
            nc.vector.memset(ones_sb[:], 1.0)
            eps_b = pp.tile([P, 1], F32)
            nc.vector.memset(eps_b[:], 1e-24)

            w_sb = pp.tile([P, nkt * d], BF16)
            nc.sync.dma_start(
                out=w_sb[:].rearrange("p (a q) -> p a q", q=d),
                in_=w_t[:].rearrange("(a p) q -> p a q", p=P),
            )
            cw_sb = pp.tile([P, 3 * nclass], BF16)
            nc.sync.dma_start(out=cw_sb[:], in_=cw_t[:])
            src_sb = pp.tile([P, T], I32)
            nc.sync.dma_start(out=src_sb[:], in_=src_t[:])

            hn = pp.tile([P, nchunks * d], BF16)  # normalized features (own nodes)
            ag_in = dp.tile([npc_pad, d], BF16)

            # ---------------- P0: pca + lrelu + l2norm + sub-allgathers ----
            qnext = 0
            for m in range(nchunks):
                xt = sb.tile([P, nkt * P], BF16, tag="xt", bufs=3)
                nc.sync.dma_start(
                    out=xt[:].rearrange("p (a q) -> p a q", q=P),
                    in_=xT_t[:, m * P:(m + 1) * P].rearrange("(a p) q -> p a q", p=P),
                )
                h_ps = put.tile([P, 1024], F32, space="PSUM", tag="ut")
                for a in range(nkt):
                    nc.tensor.matmul(
                        out=h_ps[:, :d],
                        lhsT=xt[:, a * P:(a + 1) * P],
                        rhs=w_sb[:, a * d:(a + 1) * d],
                        start=(a == 0),
                        stop=(a == nkt - 1),
                    )
                hs = sb.tile([P, d], F32, tag="hs")
                nc.vector.tensor_scalar_mul(out=hs[:], in0=h_ps[:, :d], scalar1=SLOPE)
                h = sb.tile([P, d], F32, tag="h")
                nc.vector.tensor_tensor(out=h[:], in0=h_ps[:, :d], in1=hs[:], op=OP.max)
                sq = sb.tile([P, d], F32, tag="sq")
                nc.scalar.activation(out=sq[:], in_=h[:], func=AF.Square)
                ss = sb.tile([P, K], F32, tag="ss")
                nc.vector.reduce_sum(
                    out=ss[:], in_=sq[:].rearrange("p (k e) -> p k e", k=K),
                    axis=AX.X,
                )
                lg = sb.tile([P, K], F32, tag="lg")
                nc.scalar.activation(out=lg[:], in_=ss[:], func=AF.Ln, bias=eps_b[:, :1])
                rr = sb.tile([P, K], F32, tag="rr")
                nc.scalar.activation(out=rr[:], in_=lg[:], func=AF.Exp, scale=-0.5)
                nc.vector.tensor_tensor(
                    out=hn[:, m * d:(m + 1) * d].rearrange("p (k e) -> p k e", k=K),
                    in0=h[:].rearrange("p (k e) -> p k e", k=K),
                    in1=rr[:].unsqueeze(2).to_broadcast([P, K, dd]),
                    op=OP.mult,
                )
                nc.sync.dma_start(
                    out=ag_in[m * P:(m + 1) * P, :], in_=hn[:, m * d:(m + 1) * d]
                )
                if m == bounds[qnext + 1] - 1:
                    q = qnext
                    nc.gpsimd.collective_compute(
                        "AllGather",
                        OP.bypass,
                        replica_groups=[list(range(n_cores))],
                        ins=[ag_in[bounds[q] * P:bounds[q + 1] * P, :]],
                        outs=[Hp.ap()[int(hq_base[q]):int(hq_base[q + 1]), :]],
                    )
                    qnext += 1

            # ---------------- P3: routing ---------------------------------
            def chunk_prologue(j):
                t0, ntj = int(toc[j]), int(nt[j])
                S_sb = sm.tile([P, max_nt * P], FP8, tag=f"S{j % 3}")
                nc.sync.dma_start(
                    out=S_sb[:, :ntj * P], in_=S_t[:, t0 * P:(t0 + ntj) * P]
                )
                ST_sb = sm.tile([P, max_nt * P], FP8, tag=f"ST{j % 3}")
                nc.sync.dma_start(
                    out=ST_sb[:, :ntj * P], in_=ST_t[:, t0 * P:(t0 + ntj) * P]
                )
                zch = sz.tile([P, max_nt * d], BF16, tag="z")
                for b0 in range(0, ntj, ZBATCH):
                    bn = min(ZBATCH, ntj - b0)
                    nc.gpsimd.indirect_dma_start(
                        out=zch[:, b0 * d:(b0 + bn) * d],
                        out_offset=None,
                        in_=Hp.ap(),
                        in_offset=bass.IndirectOffsetOnAxis(
                            ap=src_sb[:, t0 + b0:t0 + b0 + bn], axis=0
                        ),
                    )
                return dict(j=j, ntj=ntj, zch=zch, S_sb=S_sb, ST_sb=ST_sb,
                            u_j=None, plan=_group_plan(ntj))

            def emit_group(st, it, plan_entry):
                j, zch, S_sb, ST_sb, seg = (
                    st["j"], st["zch"], st["S_sb"], st["ST_sb"], st["seg"])
                hn_j = hn[:, j * d:(j + 1) * d]
                u_rhs = hn_j if it == 0 else st["u_j"][:]
                ti = st["ti"]
                for (g0, gn, spans, (nfull, rem)) in [plan_entry]:
                    utp = put.tile([P, 1024], F32, space="PSUM", tag="ut")
                    for i, t in enumerate(range(g0, g0 + gn)):
                        nc.tensor.matmul(
                            out=utp[:, spans[i]:spans[i] + d],
                            lhsT=ST_sb[:, t * P:(t + 1) * P],
                            rhs=u_rhs,
                            start=True, stop=True,
                        )
                    utb = sr.tile([P, GT * d], BF16, tag="utb")
                    if nfull:
                        nc.scalar.copy(
                            out=utb[:, :nfull * 3 * d],
                            in_=utp[:, :nfull * 512].rearrange(
                                "p (b x) -> p b x", b=nfull)[:, :, :3 * d],
                        )
                    if rem:
                        nc.scalar.copy(
                            out=utb[:, nfull * 3 * d:gn * d],
                            in_=utp[:, nfull * 512:nfull * 512 + rem * d],
                        )
                    zg = zch[:, g0 * d:(g0 + gn) * d]
                    prod = sr.tile([P, GT * d], BF16, tag="prod")
                    nc.vector.tensor_mul(out=prod[:, :gn * d], in0=zg, in1=utb[:, :gn * d])
                    pv = prod[:, :gn * d].rearrange("p (a e) -> p a e", e=dd)
                    t1 = sr.tile([P, GT * d // 2], BF16, tag="t1")
                    nc.vector.tensor_add(
                        out=t1[:, :gn * d // 2].rearrange("p (a e) -> p a e", e=8),
                        in0=pv[:, :, 0:8], in1=pv[:, :, 8:16],
                    )
                    t1v = t1[:, :gn * d // 2].rearrange("p (a e) -> p a e", e=8)
                    t2 = sr.tile([P, GT * d // 4], BF16, tag="t2")
                    nc.vector.tensor_add(
                        out=t2[:, :gn * d // 4].rearrange("p (a e) -> p a e", e=4),
                        in0=t1v[:, :, 0:4], in1=t1v[:, :, 4:8],
                    )
                    t2v = t2[:, :gn * d // 4].rearrange("p (a e) -> p a e", e=4)
                    t3 = sr.tile([P, GT * d // 8], BF16, tag="t3")
                    nc.vector.tensor_add(
                        out=t3[:, :gn * d // 8].rearrange("p (a e) -> p a e", e=2),
                        in0=t2v[:, :, 0:2], in1=t2v[:, :, 2:4],
                    )
                    t3v = t3[:, :gn * d // 8].rearrange("p (a e) -> p a e", e=2)
                    sf = sr.tile([P, GT * K], F32, tag="sf")
                    nc.vector.tensor_add(
                        out=sf[:, :gn * K],
                        in0=t3v[:, :, 0:1].squeeze(2), in1=t3v[:, :, 1:2].squeeze(2),
                    )
                    ef = sr.tile([P, GT * K], BF16, tag="ef")
                    nc.scalar.activation(out=ef[:, :gn * K], in_=sf[:, :gn * K], func=AF.Exp)
                    qf = sr.tile([P, GT], F32, tag="qf")
                    nc.vector.reduce_sum(
                        out=qf[:, :gn],
                        in_=ef[:, :gn * K].rearrange("p (a k) -> p a k", k=K),
                        axis=AX.X,
                    )
                    rf = sr.tile([P, GT], F32, tag="rf")
                    nc.vector.reciprocal(out=rf[:, :gn], in_=qf[:, :gn])
                    pe2 = sr.tile([P, GT * K * 2], BF16, tag="pe2")
                    p2v = pe2[:].rearrange("p (a k two) -> p a k two", k=K, two=2)
                    efv = ef[:, :gn * K].rearrange("p (a k) -> p a k", k=K)
                    rfv = rf[:, :gn].unsqueeze(2).to_broadcast([P, gn, K])
                    nc.vector.tensor_tensor(
                        out=p2v[:, :gn, :, 0:1].squeeze(3), in0=efv, in1=rfv, op=OP.mult)
                    nc.vector.tensor_tensor(
                        out=p2v[:, :gn, :, 1:2].squeeze(3), in0=efv, in1=rfv, op=OP.mult)
                    pex = sr.tile([P, GT * d], BF16, tag="pex")
                    nc.scalar.copy(
                        out=pex.bitcast(F32)[:, :gn * d // 2].rearrange(
                            "p (a e) -> p a e", e=dd // 2),
                        in_=pe2.bitcast(F32)[:, :gn * K].unsqueeze(2).to_broadcast(
                            [P, gn * K, dd // 2]),
                    )
                    msg = sr.tile([P, GT * d], BF16, tag="msg")
                    nc.vector.tensor_mul(out=msg[:, :gn * d], in0=zg, in1=pex[:, :gn * d])
                    for i, t in enumerate(range(g0, g0 + gn)):
                        nc.tensor.matmul(
                            out=seg,
                            lhsT=S_sb[:, t * P:(t + 1) * P],
                            rhs=msg[:, i * d:(i + 1) * d],
                            start=(ti == 0), stop=False,
                        )
                        ti += 1
                st["ti"] = ti

            def chunk_residual(st):
                # + x residual via identity matmul, closes the accumulation
                nc.tensor.matmul(
                    out=st["seg"], lhsT=ident[:],
                    rhs=hn[:, st["j"] * d:(st["j"] + 1) * d],
                    start=False, stop=True)

            def chunk_epilogue(st, it):
                j, seg = st["j"], st["seg"]
                sq2 = se.tile([P, d], F32, tag="sq2")
                nc.scalar.activation(out=sq2[:], in_=seg, func=AF.Square)
                ss2 = se.tile([P, K], F32, tag="ss2")
                nc.vector.reduce_sum(
                    out=ss2[:], in_=sq2[:].rearrange("p (k e) -> p k e", k=K),
                    axis=AX.X,
                )
                lg2 = se.tile([P, K], F32, tag="lg2")
                nc.scalar.activation(out=lg2[:], in_=ss2[:], func=AF.Ln, bias=eps_b[:, :1])
                rr2 = se.tile([P, K], F32, tag="rr2")
                nc.scalar.activation(out=rr2[:], in_=lg2[:], func=AF.Exp, scale=-0.5)
                rrb = rr2[:].unsqueeze(2).to_broadcast([P, K, dd])
                segv = seg.rearrange("p (k e) -> p k e", k=K)
                if it < NITER - 1:
                    u_new = se.tile([P, d], BF16, tag=f"uj{j % 3}", bufs=1)
                    nc.vector.tensor_tensor(
                        out=u_new[:].rearrange("p (k e) -> p k e", k=K),
                        in0=segv, in1=rrb, op=OP.mult)
                    st["u_j"] = u_new
                else:
                    uf = se.tile([P, d], F32, tag="uf")
                    nc.vector.tensor_tensor(
                        out=uf[:].rearrange("p (k e) -> p k e", k=K),
                        in0=segv, in1=rrb, op=OP.mult)
                    us = se.tile([P, d], F32, tag="us")
                    nc.vector.tensor_scalar_mul(out=us[:], in0=uf[:], scalar1=SLOPE)
                    ufb = se.tile([P, d], BF16, tag="ufb")
                    nc.vector.tensor_tensor(out=ufb[:], in0=uf[:], in1=us[:], op=OP.max)
                    trp = ptr.tile([P, 1024], BF16, space="PSUM", tag="tr")
                    nc.tensor.transpose(out=trp[:, :P], in_=ufb[:, :P], identity=ident[:])
                    nc.tensor.transpose(
                        out=trp[:d - P, P:2 * P], in_=ufb[:, P:d], identity=ident[:])
                    uT = se.tile([P, 2 * P], BF16, tag="uT")
                    nc.scalar.copy(out=uT[:, :P], in_=trp[:, :P])
                    nc.scalar.copy(out=uT[:d - P, P:], in_=trp[:d - P, P:2 * P])
                    yp = ptr.tile([P, 1024], BF16, space="PSUM", tag="tr")
                    ypv = yp.bitcast(F32)[:, :nclass]
                    nc.tensor.matmul(
                        out=ypv, lhsT=uT[:, :P], rhs=cw_sb[:, :nclass],
                        start=True, stop=False)
                    nc.tensor.matmul(
                        out=ypv, lhsT=uT[:d - P, P:2 * P],
                        rhs=cw_sb[:d - P, nclass:2 * nclass],
                        start=False, stop=False)
                    nc.tensor.matmul(
                        out=ypv, lhsT=ones_sb[:, :P], rhs=cw_sb[0:1, 2 * nclass:],
                        start=False, stop=True)
                    ysb = se.tile([P, nclass], F32, tag="ysb")
                    nc.scalar.copy(out=ysb[:], in_=ypv)
                    nc.sync.dma_start(out=y_t[j * P:(j + 1) * P, :], in_=ysb[:])

            # triples, but finish with two pairs so the pipeline drain after
            # the last z-gather is short
            if nchunks > 7 and nchunks % 3 == 1:
                ntrip = (nchunks - 4) // 3
                cgroups = [list(range(j0, j0 + 3))
                           for j0 in range(0, 3 * ntrip, 3)]
                cgroups += [[nchunks - 4, nchunks - 3], [nchunks - 2, nchunks - 1]]
            else:
                cgroups = [list(range(j0, min(j0 + 3, nchunks)))
                           for j0 in range(0, nchunks, 3)]
            for cg in cgroups:
                sts = [chunk_prologue(j) for j in cg]
                for it in range(NITER):
                    for st in sts:
                        segt = pse.tile([P, 512], F32, space="PSUM",
                                        tag=f"seg{st['j'] % 3}")
                        st["seg"] = segt[:, 0:d]
                        st["ti"] = 0
                    ng = max(len(st["plan"]) for st in sts)
                    for g in range(ng):
                        for st in sts:
                            if g < len(st["plan"]):
                                emit_group(st, it, st["plan"][g])
                    for st in sts:
                        chunk_residual(st)
                    for st in sts:
                        chunk_epilogue(st, it)
    return nc


_CACHE = {}
TRACE = False
LAST_RESULTS = None


def kernel(x, edge_index, pca_w, pca_b, clf_w, clf_b, n_cores=8, _sim=False):
    x = np.asarray(x, np.float32)
    edge_index = np.asarray(edge_index)
    pca_w = np.asarray(pca_w, np.float32)
    pca_b = np.asarray(pca_b, np.float32)
    clf_w = np.asarray(clf_w, np.float32)
    clf_b = np.asarray(clf_b, np.float32)

    n, nfeat = x.shape
    d = pca_w.shape[1]
    nclass = clf_w.shape[1]

    meta, src_dev, S_dev, ST_dev, xT = _host_prep(x, edge_index, n_cores)

    key = (n, nfeat, d, nclass, tuple(meta["nt"].tolist()))
    if key not in _CACHE:
        _CACHE[key] = build_program(nfeat, d, nclass, meta, n_cores)
        if not _sim:
            _split_multiwaits(_CACHE[key])
    nc = _CACHE[key]

    kf_pad = meta["kf_pad"]
    w_pad = np.zeros((kf_pad, d), ml_dtypes.bfloat16)
    w_pad[:nfeat] = pca_w.astype(ml_dtypes.bfloat16)
    w_pad[nfeat] = pca_b.astype(ml_dtypes.bfloat16)
    cwp = np.zeros((P, 3 * nclass), ml_dtypes.bfloat16)
    cwp[:, :nclass] = clf_w[:P].astype(ml_dtypes.bfloat16)
    cwp[:d - P, nclass:2 * nclass] = clf_w[P:].astype(ml_dtypes.bfloat16)
    cwp[0, 2 * nclass:] = clf_b.astype(ml_dtypes.bfloat16)

    in_maps = []
    for c in range(n_cores):
        in_maps.append({
            "xT": xT[c],
            "wp": w_pad,
            "cwp": cwp,
            "src": src_dev[c],
            "Smask": S_dev[c],
            "STmask": ST_dev[c],
        })

    npc = meta["npc"]
    npc_pad = meta["npc_pad"]
    if _sim:
        from concourse.bass_interp import CoreSim
        assert n_cores == 1
        sim = CoreSim(nc)
        for kk, vv in in_maps[0].items():
            sim.tensor(kk)[:] = vv
        sim.simulate()
        y_dev = np.asarray(sim.tensor("y"))[None]
    else:
        global LAST_RESULTS
        res = run_bass_kernel_spmd(
            nc, in_maps, core_ids=list(range(n_cores)), trace=TRACE
        )
        LAST_RESULTS = res
        y_dev = np.stack([res.results[c]["y"] for c in range(n_cores)], axis=0)

    # un-permute: node nd lives at (core, pos)
    y = np.empty((n, nclass), np.float32)
    y[np.arange(n)] = y_dev[meta["node_core"], meta["pos_in_core"]]
    return y.astype(np.float32)


if __name__ == "__main__":
    import pickle, time
    with open("/tmp/ref_inputs.pkl", "rb") as f:
        inputs = pickle.load(f)
    t0 = time.time()
    y = kernel(**inputs)
    print("kernel() wall time", time.time() - t0)
    np.save("/tmp/kernel_out.npy", y)
